# revision 28
# baseline (speedup 1.0000x reference)
"""Trainium2 Bass kernel for nn_BaselinePhasorBlock (B=2, L=1024, D=512, K=64).

Algorithm restructure: the phasor-memory cumsum
    retrieved[t,d] = Re[ sum_k e^{-i q[t,k]} * sum_{s<=t} e^{i key[s,k]} v[s,d] ]
collapses to causal attention:
    A[t,s] = cosQ[t]·cosK[s] + sinQ[t]·sinK[s]   (dot over k)
    retrieved = tril(A) @ value
so nothing of size (L,K,D) is ever materialized.

LayerNorm folding (exact):
    LN(retrieved/norm) @ Wo + bo + x
  = scale_t * (r @ Wg - mu_t * cw) + [x + ln_b@Wo + bo]
with Wg = diag(ln_g)@Wo, cw = colsums(Wg), scale_t = 1/sqrt(var_r + eps*norm_t^2),
norm_t^2 = (t+1)*K.  (LN row stats are scale-invariant up to the eps term, which
is folded into the eps exactly.)

Sharding (8 cores, SPMD, no collectives): core c -> batch b = c//4, strip pair
i = c%4 owning t-strips [i*128, (i+1)*128) and [(7-i)*128, (8-i)*128).  The
pairing makes causal work uniform; causality is enforced by a per-core mask on
the score matrix (AT layout [s, t]).  Each core computes its batch's keys and
values over the full sequence (redundant across the 4 cores of a batch, but
avoids collectives entirely).

All matmuls run in bf16 (validated: ~1.7e-3 scale-relative error end-to-end
vs the fp32 jax reference), fp32 PSUM accumulation, fp32 residual/output path.

Inputs are host-packed into a handful of layout-exact mega-tensors so the
kernel issues only ~6 DMAs, ordered by first use (DMA-issue serialization on
the sync sequencer was the dominant stall in v1).
"""

import math
from contextlib import ExitStack

import numpy as np

B, L, D, K = 2, 1024, 512, 64
PI = math.pi
NCORES = 8
NSC = L // 128  # 8 s-chunks
NDC = D // 128  # 4 d-chunks
EPS = 1e-5

# flat offsets (in elements) inside the packed DMA groups
EARLY_BF = {"wk1": (0, 2048), "wk2d": (2048, 512), "xTa": (2560, 2048),
            "xTb": (4608, 2048)}
EARLY_BF_W = 6656
MID_BF = {"wq1": (0, 2048), "qxT": (2048, 1024),
          "wq2d": (3072, 512), "wv": (3584, 2048)}
MID_BF_W = 5632
F32A = {"bk1": (0, 4), "bq1": (4, 4), "bk2d": (8, 1), "bq2d": (9, 1),
        "bvb": (10, 512)}
F32A_W = 522
LATE_BF = {"mask": (0, 2048), "wg": (2048, 2048)}
LATE_BF_W = 4096
F32B = {"xplus": (0, 1024), "epsn2": (1024, 2)}
F32B_W = 1026

_CACHE = {}


def _build_program(gelu_override=None):
    import concourse.bacc as bacc
    import concourse.mybir as mybir
    import concourse.tile as tile

    AF = mybir.ActivationFunctionType
    ALU = mybir.AluOpType
    GELU = AF.Gelu if gelu_override is None else gelu_override
    FP32 = mybir.dt.float32
    BF16 = mybir.dt.bfloat16

    nc = bacc.Bacc()

    d_early = nc.declare_dram_parameter("early_bf", [128, EARLY_BF_W], BF16, False)
    d_f32a = nc.declare_dram_parameter("f32a", [128, F32A_W], FP32, False)
    d_mid = nc.declare_dram_parameter("mid_bf", [128, MID_BF_W], BF16, False)
    d_late = nc.declare_dram_parameter("late_bf", [128, LATE_BF_W], BF16, False)
    d_f32b = nc.declare_dram_parameter("f32b", [128, F32B_W], FP32, False)
    d_cw = nc.declare_dram_parameter("cw", [1, D], BF16, False)
    d_out = nc.declare_dram_parameter("out", [2, 128, D], FP32, True)

    with tile.TileContext(nc) as tc, ExitStack() as ctx:
        consts = ctx.enter_context(tc.tile_pool(name="consts", bufs=1))
        work = ctx.enter_context(tc.tile_pool(name="work", bufs=1))
        atm_pool = ctx.enter_context(tc.tile_pool(name="atm", bufs=3))
        small = ctx.enter_context(tc.tile_pool(name="small", bufs=1))
        ps_big = ctx.enter_context(tc.tile_pool(name="ps_big", bufs=2, space="PSUM"))
        ps_at = ctx.enter_context(tc.tile_pool(name="ps_at", bufs=2, space="PSUM"))
        ps_rt = ctx.enter_context(tc.tile_pool(name="ps_rt", bufs=1, space="PSUM"))

        # ---- SBUF tiles for the packed input groups + views ----
        early = consts.tile([128, EARLY_BF_W], BF16)
        mid = consts.tile([128, MID_BF_W], BF16)
        f32a = consts.tile([128, F32A_W], FP32)
        late = consts.tile([128, LATE_BF_W], BF16)
        f32b = consts.tile([128, F32B_W], FP32)
        cw = consts.tile([1, D], BF16)
        ones = consts.tile([128, 1], BF16)
        cosbias = consts.tile([128, 1], FP32)
        sinscale = consts.tile([128, 1], FP32)

        def view(tile_, table, name, c=None):
            off, w = table[name]
            v = tile_[:, off:off + w]
            if c is not None:
                v = v.rearrange("p (c f) -> p c f", c=c)
            return v

        wk1 = view(early, EARLY_BF, "wk1", 4)     # [128, 4, 512]
        wk2d = view(early, EARLY_BF, "wk2d", 4)   # [128, 4, 128]
        xTa = view(early, EARLY_BF, "xTa", 4)     # [128, 4, 512]
        xTb = view(early, EARLY_BF, "xTb", 4)
        wq1 = view(mid, MID_BF, "wq1", 4)
        qxT = view(mid, MID_BF, "qxT", 4)         # [128, 4, 256]
        wq2d = view(mid, MID_BF, "wq2d", 4)
        wv = view(mid, MID_BF, "wv", 4)
        bk1 = view(f32a, F32A, "bk1")             # [128, 4]
        bq1 = view(f32a, F32A, "bq1")
        bk2d = view(f32a, F32A, "bk2d")           # [128, 1]
        bq2d = view(f32a, F32A, "bq2d")
        bvb = view(f32a, F32A, "bvb")             # [128, 512]
        maskt = view(late, LATE_BF, "mask", NSC)  # [128, 8, 256]
        wg = view(late, LATE_BF, "wg", 4)
        xplus = view(f32b, F32B, "xplus", 2)      # [128, 2, 512]
        epsn2 = view(f32b, F32B, "epsn2")         # [128, 2]

        def xT(c, lo, hi):
            """x^T slice [din-chunk c, seq cols lo:hi] across the a/b halves."""
            if hi <= 512:
                return xTa[:, c, lo:hi]
            return xTb[:, c, lo - 512:hi - 512]

        # ---- DMAs: one per group, issued in need-order on sync ----
        nc.sync.dma_start(out=early, in_=d_early[:])
        nc.sync.dma_start(out=f32a, in_=d_f32a[:])
        nc.sync.dma_start(out=mid, in_=d_mid[:])
        nc.sync.dma_start(out=late, in_=d_late[:])
        nc.sync.dma_start(out=f32b, in_=d_f32b[:])
        nc.sync.dma_start(out=cw, in_=d_cw[:])
        nc.vector.memset(ones, 1.0)
        nc.vector.memset(cosbias[0:64, :], PI / 2)
        nc.vector.memset(cosbias[64:128, :], 0.0)
        nc.vector.memset(sinscale[0:64, :], -PI)
        nc.vector.memset(sinscale[64:128, :], PI)

        # ---- working SBUF tiles ----
        hkT = work.tile([128, 4, L], BF16)      # gelu(x@Wk1+b) transposed
        hqT = work.tile([128, 4, 256], BF16)
        kph2 = work.tile([128, L], BF16)        # tanh phase, duplicated halves
        qph2 = work.tile([128, 256], BF16)
        KS = work.tile([128, L], BF16)          # rows 0:64 cosK, 64:128 sinK
        QS = work.tile([128, 256], BF16)
        value = work.tile([128, NSC, D], BF16)  # value rows [s,d] per s-chunk
        rT_sb = work.tile([128, 4, 256], BF16)  # retrievedT [d, t]
        rsq = work.tile([128, 4, 256], BF16)
        out_sb = work.tile([128, 2, D], FP32)

        # ---- MLP1 (key): hkT[j, :] = gelu(Wk1^T @ xT + bk1) ----
        # One 2-bank PSUM tile per j; m-halves are separate accumulation
        # groups (one start per bank), one big gelu over both.
        for j in range(4):
            ps = ps_big.tile([128, 1024], FP32, tag="mlp")
            for m in range(2):
                for c in range(4):
                    nc.tensor.matmul(
                        ps[:, m * 512:(m + 1) * 512],
                        lhsT=wk1[:, c, j * 128:(j + 1) * 128],
                        rhs=xT(c, m * 512, (m + 1) * 512),
                        start=(c == 0),
                        stop=(c == 3),
                    )
            nc.scalar.activation(
                out=hkT[:, j, :], in_=ps,
                func=GELU, bias=bk1[:, j:j + 1], scale=1.0,
            )

        # ---- MLP1 (query): 4 j-groups packed into one 2-bank tile ----
        ps_q = ps_big.tile([128, 1024], FP32, tag="mlp")
        for j in range(4):
            reg = ps_q[:, j * 256:(j + 1) * 256]
            for c in range(4):
                nc.tensor.matmul(
                    reg,
                    lhsT=wq1[:, c, j * 128:(j + 1) * 128],
                    rhs=qxT[:, c, :],
                    start=(c == 0 and j % 2 == 0),  # one start per bank
                    stop=(c == 3 and j % 2 == 1),
                )
        for j in range(4):  # per-j gelu (per-partition bias differs per j)
            nc.scalar.activation(
                out=hqT[:, j, :],
                in_=ps_q[:, j * 256:(j + 1) * 256],
                func=GELU, bias=bq1[:, j:j + 1], scale=1.0,
            )

        # ---- value rows: value[s, d] = x@Wv + bv (two s-chunks per tile) ----
        for sp in range(NSC // 2):
            ps = ps_big.tile([128, 1024], FP32, tag="mlp")
            for h in range(2):
                sc = 2 * sp + h
                for c in range(4):
                    nc.tensor.matmul(
                        ps[:, h * 512:(h + 1) * 512],
                        lhsT=xT(c, sc * 128, (sc + 1) * 128),
                        rhs=wv[:, c, :],
                        start=(c == 0),
                        stop=(c == 3),
                    )
            for h in range(2):
                sc = 2 * sp + h
                nc.vector.tensor_add(out=value[:, sc, :],
                                     in0=ps[:, h * 512:(h + 1) * 512], in1=bvb)

        # ---- key phase matmul + tanh (duplicated halves via doubled Wk2) ----
        ps_k = ps_big.tile([128, 1024], FP32, tag="mlp")
        for m in range(2):
            for j in range(4):
                nc.tensor.matmul(
                    ps_k[:, m * 512:(m + 1) * 512],
                    lhsT=wk2d[:, j, :],
                    rhs=hkT[:, j, m * 512:(m + 1) * 512],
                    start=(j == 0),
                    stop=(j == 3),
                )
        nc.scalar.activation(out=kph2, in_=ps_k, func=AF.Tanh,
                             bias=bk2d, scale=1.0)
        # ---- query phase matmul + tanh ----
        ps_p = ps_big.tile([128, 1024], FP32, tag="mlp")
        for j in range(4):
            nc.tensor.matmul(
                ps_p[:, 0:256],
                lhsT=wq2d[:, j, :],
                rhs=hqT[:, j, :],
                start=(j == 0),
                stop=(j == 3),
            )
        nc.scalar.activation(out=qph2, in_=ps_p[:, 0:256], func=AF.Tanh,
                             bias=bq2d, scale=1.0)

        # ---- |t| on the cos half (ACT Abs; in every table set) ----
        nc.scalar.activation(out=kph2[0:64, :], in_=kph2[0:64, :], func=AF.Abs)
        nc.scalar.activation(out=qph2[0:64, :], in_=qph2[0:64, :], func=AF.Abs)

        # ---- cos/sin of phases (stacked halves: 0:64 cos, 64:128 sin) ----
        # ACT Sin domain is [-pi, pi]; with t in (-1,1):
        #   cos(pi t) = sin(pi/2 - pi |t|),  sin(pi t) = sin(pi t)
        # one Sin pass with per-partition scale (-pi / +pi) and bias (pi/2 / 0).
        nc.scalar.activation(out=KS, in_=kph2, func=AF.Sin,
                             bias=cosbias, scale=sinscale)
        nc.scalar.activation(out=QS, in_=qph2, func=AF.Sin,
                             bias=cosbias, scale=sinscale)

        # ---- scores + causal mask + retrievedT accumulation ----
        rt_ps = ps_rt.tile([128, 4, 256], FP32)
        for sc in range(NSC):
            at_ps = ps_at.tile([128, 256], FP32, tag="at")
            nc.tensor.matmul(
                at_ps,
                lhsT=KS[:, sc * 128:(sc + 1) * 128],
                rhs=QS,
                start=True,
                stop=True,
            )
            atm = atm_pool.tile([128, 256], BF16, tag="atm")
            nc.vector.tensor_mul(out=atm, in0=at_ps, in1=maskt[:, sc, :])
            # rt_ps spans 2 banks (dc 0,1 | dc 2,3): exactly one start per
            # bank (first matmul into it), one stop on each bank's last.
            for dc in range(NDC):
                nc.tensor.matmul(
                    rt_ps[:, dc, :],
                    lhsT=value[:, sc, dc * 128:(dc + 1) * 128],
                    rhs=atm,
                    start=(sc == 0 and dc in (0, 2)),
                    stop=(sc == NSC - 1 and dc in (1, 3)),
                )

        # ---- retrievedT -> SBUF (ACT, idle by now) + squares (DVE) ----
        for dc in range(NDC):
            nc.scalar.copy(out=rT_sb[:, dc, :], in_=rt_ps[:, dc, :])
        for dc in range(NDC):
            nc.vector.tensor_mul(out=rsq[:, dc, :], in0=rT_sb[:, dc, :],
                                 in1=rT_sb[:, dc, :])

        # ---- row stats: sums/sumsq share one bank (one group); row sums
        # reuse the at-pool bank (free by now) with their own group ----
        sums_ps = ps_at.tile([128, 4], FP32, tag="at")  # [sum s0, sum s1, sq s0, sq s1]
        row_ps = ps_at.tile([1, 256], FP32, tag="at")
        first = True
        n = 0
        for st in range(2):
            for src, col in ((rT_sb, st), (rsq, 2 + st)):
                for dc in range(NDC):
                    n += 1
                    nc.tensor.matmul(
                        sums_ps[:, col:col + 1],
                        lhsT=src[:, dc, st * 128:(st + 1) * 128],
                        rhs=ones,
                        start=first,
                        stop=(n == 16),
                    )
                    first = False
        for dc in range(NDC):
            nc.tensor.matmul(
                row_ps,
                lhsT=ones,
                rhs=rT_sb[:, dc, :],
                start=(dc == 0),
                stop=(dc == 3),
            )

        # negmu_row = -(row sums)/D  (bf16, feeds the rank-1 mean-fold matmul)
        negmu = small.tile([1, 256], BF16)
        nc.vector.tensor_scalar_mul(out=negmu, in0=row_ps, scalar1=-1.0 / D)

        # per-strip scale_t = 1/sqrt(var + eps*norm^2)
        mu = small.tile([128, 2], FP32)
        musq = small.tile([128, 2], FP32)
        var = small.tile([128, 2], FP32)
        scl = small.tile([128, 2], FP32)
        for st in range(2):
            nc.vector.tensor_scalar_mul(out=mu[:, st:st + 1],
                                        in0=sums_ps[:, st:st + 1],
                                        scalar1=1.0 / D)
            nc.vector.tensor_mul(out=musq[:, st:st + 1],
                                 in0=mu[:, st:st + 1], in1=mu[:, st:st + 1])
            nc.vector.scalar_tensor_tensor(
                out=var[:, st:st + 1],
                in0=sums_ps[:, 2 + st:3 + st],
                scalar=1.0 / D,
                in1=musq[:, st:st + 1],
                op0=ALU.mult,
                op1=ALU.subtract,
            )
        for st in range(2):
            nc.scalar.activation(out=scl[:, st:st + 1], in_=var[:, st:st + 1],
                                 func=AF.Sqrt, bias=epsn2[:, st:st + 1],
                                 scale=1.0)
            nc.vector.reciprocal(out=scl[:, st:st + 1], in_=scl[:, st:st + 1])

        # ---- output: out = scale * (rT^T @ Wg - mu*cw) + xplus ----
        for st in range(2):
            ps = ps_big.tile([128, 1024], FP32, tag="mlp")
            reg = ps[:, 0:512]
            for dc in range(NDC):
                nc.tensor.matmul(
                    reg,
                    lhsT=rT_sb[:, dc, st * 128:(st + 1) * 128],
                    rhs=wg[:, dc, :],
                    start=(dc == 0),
                    stop=False,
                )
            nc.tensor.matmul(
                reg,
                lhsT=negmu[:, st * 128:(st + 1) * 128],
                rhs=cw,
                start=False,
                stop=True,
            )
            nc.vector.scalar_tensor_tensor(
                out=out_sb[:, st, :],
                in0=reg,
                scalar=scl[:, st:st + 1],
                in1=xplus[:, st, :],
                op0=ALU.mult,
                op1=ALU.add,
            )
            nc.sync.dma_start(out=d_out[st], in_=out_sb[:, st, :])

    return nc


def _host_prepare(inputs):
    """Build the 8 per-core input maps (host-side numpy packing)."""
    import ml_dtypes

    bf16 = ml_dtypes.bfloat16
    f32 = np.float32

    x = np.asarray(inputs["x"], f32)
    Wk1 = np.asarray(inputs["Wk1"], f32)
    bk1 = np.asarray(inputs["bk1"], f32)
    Wk2 = np.asarray(inputs["Wk2"], f32)
    bk2 = np.asarray(inputs["bk2"], f32)
    Wq1 = np.asarray(inputs["Wq1"], f32)
    bq1 = np.asarray(inputs["bq1"], f32)
    Wq2 = np.asarray(inputs["Wq2"], f32)
    bq2 = np.asarray(inputs["bq2"], f32)
    Wv = np.asarray(inputs["Wv"], f32)
    bv = np.asarray(inputs["bv"], f32)
    ln_g = np.asarray(inputs["ln_g"], f32)
    ln_b = np.asarray(inputs["ln_b"], f32)
    Wo = np.asarray(inputs["Wo"], f32)
    bo = np.asarray(inputs["bo"], f32)

    Wg32 = ln_g[:, None] * Wo
    cw = Wg32.astype(bf16).astype(f32).sum(axis=0).astype(bf16).reshape(1, D)
    out_bias = (ln_b @ Wo + bo).astype(f32)

    def pack(w):  # [D_in, F] -> [128, 4, F]
        return w.reshape(4, 128, -1).transpose(1, 0, 2)

    wk2d = np.concatenate([Wk2, Wk2], axis=1)  # [512, 128]
    wq2d = np.concatenate([Wq2, Wq2], axis=1)

    def fill(width, table, parts, dt):
        buf = np.zeros((128, width), dt)
        for name, arr in parts.items():
            off, w = table[name]
            buf[:, off:off + w] = arr.reshape(128, w).astype(dt)
        return buf

    early_base = {"wk1": pack(Wk1), "wk2d": pack(wk2d)}
    mid_base = {"wq1": pack(Wq1), "wq2d": pack(wq2d), "wv": pack(Wv)}
    f32a_arr = fill(F32A_W, F32A, {
        "bk1": bk1.reshape(4, 128).T,
        "bq1": bq1.reshape(4, 128).T,
        "bk2d": np.concatenate([bk2, bk2]).reshape(128, 1),
        "bq2d": np.concatenate([bq2, bq2]).reshape(128, 1),
        "bvb": np.broadcast_to(bv, (128, D)),
    }, f32)
    wg_packed = pack(Wg32)

    in_maps = []
    for c in range(NCORES):
        b, i = divmod(c, 4)
        t0, t1 = i * 128, (7 - i) * 128
        xb = x[b]  # [L, D]
        xTp = pack(np.ascontiguousarray(xb.T))  # [128, 4, L]
        qx = np.concatenate(
            [xb[t0:t0 + 128].T, xb[t1:t1 + 128].T], axis=1)  # [512, 256]
        tglob = np.concatenate(
            [np.arange(t0, t0 + 128), np.arange(t1, t1 + 128)])
        mask = (np.arange(L)[:, None] <= tglob[None, :])  # [L, 256]
        xplus = np.stack([xb[t0:t0 + 128], xb[t1:t1 + 128]]) + out_bias
        epsn2 = (EPS * K * (tglob.astype(f32) + 1.0)).reshape(2, 128)

        m = {
            "early_bf": fill(EARLY_BF_W, EARLY_BF,
                             {**early_base, "xTa": xTp[:, :, 0:512],
                              "xTb": xTp[:, :, 512:1024]}, bf16),
            "mid_bf": fill(MID_BF_W, MID_BF,
                           {**mid_base, "qxT": pack(qx)}, bf16),
            "f32a": f32a_arr,
            "late_bf": fill(LATE_BF_W, LATE_BF, {
                "mask": mask.reshape(NSC, 128, 256).transpose(1, 0, 2),
                "wg": wg_packed,
            }, bf16),
            "f32b": fill(F32B_W, F32B, {
                "xplus": xplus.transpose(1, 0, 2),
                "epsn2": epsn2.T,
            }, f32),
            "cw": cw,
        }
        in_maps.append(m)
    return in_maps


def run(inputs, trace=False):
    from concourse.bass_utils import run_bass_kernel_spmd

    if "nc" not in _CACHE:
        nc = _build_program()
        nc.finalize()
        _CACHE["nc"] = nc
    nc = _CACHE["nc"]
    in_maps = _host_prepare(inputs)
    res = run_bass_kernel_spmd(nc, in_maps, list(range(NCORES)), trace=trace)
    out = np.empty((B, L, D), np.float32)
    for c in range(NCORES):
        b, i = divmod(c, 4)
        oc = np.asarray(res.results[c]["out"], np.float32)
        out[b, i * 128:(i + 1) * 128] = oc[0]
        out[b, (7 - i) * 128:(8 - i) * 128] = oc[1]
    return out, res


def kernel(**inputs):
    out, _ = run(inputs, trace=False)
    return out


# revision 29
# speedup vs baseline: 1.1243x; 1.1243x over previous
"""Trainium2 Bass kernel for nn_BaselinePhasorBlock (B=2, L=1024, D=512, K=64).

Algorithm restructure: the phasor-memory cumsum
    retrieved[t,d] = Re[ sum_k e^{-i q[t,k]} * sum_{s<=t} e^{i key[s,k]} v[s,d] ]
collapses to causal attention:
    A[t,s] = cosQ[t]·cosK[s] + sinQ[t]·sinK[s]   (dot over k)
    retrieved = tril(A) @ value
so nothing of size (L,K,D) is ever materialized.

LayerNorm folding (exact):
    LN(retrieved/norm) @ Wo + bo + x
  = scale_t * (r @ Wg - mu_t * cw) + [x + ln_b@Wo + bo]
with Wg = diag(ln_g)@Wo, cw = colsums(Wg), scale_t = 1/sqrt(var_r + eps*norm_t^2),
norm_t^2 = (t+1)*K.  (LN row stats are scale-invariant up to the eps term, which
is folded into the eps exactly.)

Sharding (8 cores, SPMD, no collectives): core c -> batch b = c//4, strip pair
i = c%4 owning t-strips [i*128, (i+1)*128) and [(7-i)*128, (8-i)*128).  The
pairing makes causal work uniform; causality is enforced by a per-core mask on
the score matrix (AT layout [s, t]).  Each core computes its batch's keys and
values over the full sequence (redundant across the 4 cores of a batch, but
avoids collectives entirely).

All matmuls run in bf16 (validated: ~1.7e-3 scale-relative error end-to-end
vs the fp32 jax reference), fp32 PSUM accumulation, fp32 residual/output path.

Inputs are host-packed into a handful of layout-exact mega-tensors so the
kernel issues only ~6 DMAs, ordered by first use (DMA-issue serialization on
the sync sequencer was the dominant stall in v1).
"""

import math
from contextlib import ExitStack

import numpy as np

B, L, D, K = 2, 1024, 512, 64
PI = math.pi
NCORES = 8
NSC = L // 128  # 8 s-chunks
NDC = D // 128  # 4 d-chunks
EPS = 1e-5

# flat offsets (in elements) inside the packed DMA groups
EARLY_BF = {"wk1": (0, 2048), "wk2d": (2048, 512), "xTa": (2560, 2048),
            "xTb": (4608, 2048)}
EARLY_BF_W = 6656
MID_BF = {"wq1": (0, 2048), "qxT": (2048, 1024),
          "wq2d": (3072, 512), "wv": (3584, 2048)}
MID_BF_W = 5632
F32A = {"bk1": (0, 4), "bq1": (4, 4), "bk2d": (8, 1), "bq2d": (9, 1),
        "bvb": (10, 512)}
F32A_W = 522
LATE_BF = {"mask": (0, 2048), "wg": (2048, 2048)}
LATE_BF_W = 4096
F32B = {"xplus": (0, 1024), "epsn2": (1024, 2)}
F32B_W = 1026

_CACHE = {}


def _build_program(gelu_override=None):
    import concourse.bacc as bacc
    import concourse.mybir as mybir
    import concourse.tile as tile

    AF = mybir.ActivationFunctionType
    ALU = mybir.AluOpType
    GELU = AF.Gelu if gelu_override is None else gelu_override
    FP32 = mybir.dt.float32
    BF16 = mybir.dt.bfloat16

    nc = bacc.Bacc()

    d_early = nc.declare_dram_parameter("early_bf", [128, EARLY_BF_W], BF16, False)
    d_f32a = nc.declare_dram_parameter("f32a", [128, F32A_W], FP32, False)
    d_mid = nc.declare_dram_parameter("mid_bf", [128, MID_BF_W], BF16, False)
    d_late = nc.declare_dram_parameter("late_bf", [128, LATE_BF_W], BF16, False)
    d_f32b = nc.declare_dram_parameter("f32b", [128, F32B_W], FP32, False)
    d_cw = nc.declare_dram_parameter("cw", [1, D], BF16, False)
    d_out = nc.declare_dram_parameter("out", [2, 128, D], FP32, True)

    with tile.TileContext(nc) as tc, ExitStack() as ctx:
        consts = ctx.enter_context(tc.tile_pool(name="consts", bufs=1))
        work = ctx.enter_context(tc.tile_pool(name="work", bufs=1))
        atm_pool = ctx.enter_context(tc.tile_pool(name="atm", bufs=3))
        small = ctx.enter_context(tc.tile_pool(name="small", bufs=1))
        ps_big = ctx.enter_context(tc.tile_pool(name="ps_big", bufs=2, space="PSUM"))
        ps_at = ctx.enter_context(tc.tile_pool(name="ps_at", bufs=2, space="PSUM"))
        ps_rt = ctx.enter_context(tc.tile_pool(name="ps_rt", bufs=1, space="PSUM"))

        # ---- SBUF tiles for the packed input groups + views ----
        early = consts.tile([128, EARLY_BF_W], BF16)
        mid = consts.tile([128, MID_BF_W], BF16)
        f32a = consts.tile([128, F32A_W], FP32)
        late = consts.tile([128, LATE_BF_W], BF16)
        f32b = consts.tile([128, F32B_W], FP32)
        cw = consts.tile([1, D], BF16)
        ones = consts.tile([128, 1], BF16)
        cosbias = consts.tile([128, 1], FP32)
        sinscale = consts.tile([128, 1], FP32)

        def view(tile_, table, name, c=None):
            off, w = table[name]
            v = tile_[:, off:off + w]
            if c is not None:
                v = v.rearrange("p (c f) -> p c f", c=c)
            return v

        wk1 = view(early, EARLY_BF, "wk1", 4)     # [128, 4, 512]
        wk2d = view(early, EARLY_BF, "wk2d", 4)   # [128, 4, 128]
        xTa = view(early, EARLY_BF, "xTa", 4)     # [128, 4, 512]
        xTb = view(early, EARLY_BF, "xTb", 4)
        wq1 = view(mid, MID_BF, "wq1", 4)
        qxT = view(mid, MID_BF, "qxT", 4)         # [128, 4, 256]
        wq2d = view(mid, MID_BF, "wq2d", 4)
        wv = view(mid, MID_BF, "wv", 4)
        bk1 = view(f32a, F32A, "bk1")             # [128, 4]
        bq1 = view(f32a, F32A, "bq1")
        bk2d = view(f32a, F32A, "bk2d")           # [128, 1]
        bq2d = view(f32a, F32A, "bq2d")
        bvb = view(f32a, F32A, "bvb")             # [128, 512]
        maskt = view(late, LATE_BF, "mask", NSC)  # [128, 8, 256]
        wg = view(late, LATE_BF, "wg", 4)
        xplus = view(f32b, F32B, "xplus", 2)      # [128, 2, 512]
        epsn2 = view(f32b, F32B, "epsn2")         # [128, 2]

        def xT(c, lo, hi):
            """x^T slice [din-chunk c, seq cols lo:hi] across the a/b halves."""
            if hi <= 512:
                return xTa[:, c, lo:hi]
            return xTb[:, c, lo - 512:hi - 512]

        # ---- DMAs: one per group, issued in need-order on sync ----
        nc.sync.dma_start(out=early, in_=d_early[:])
        nc.sync.dma_start(out=f32a, in_=d_f32a[:])
        nc.sync.dma_start(out=mid, in_=d_mid[:])
        nc.sync.dma_start(out=late, in_=d_late[:])
        nc.sync.dma_start(out=f32b, in_=d_f32b[:])
        nc.sync.dma_start(out=cw, in_=d_cw[:])
        nc.vector.memset(ones, 1.0)
        nc.vector.memset(cosbias[0:64, :], PI / 2)
        nc.vector.memset(cosbias[64:128, :], 0.0)
        nc.vector.memset(sinscale[0:64, :], -PI)
        nc.vector.memset(sinscale[64:128, :], PI)

        # ---- working SBUF tiles ----
        hkT = work.tile([128, 4, L], BF16)      # gelu(x@Wk1+b) transposed
        hqT = work.tile([128, 4, 256], BF16)
        kph2 = work.tile([128, L], BF16)        # tanh phase, duplicated halves
        qph2 = work.tile([128, 256], BF16)
        KS = work.tile([128, L], BF16)          # rows 0:64 cosK, 64:128 sinK
        QS = work.tile([128, 256], BF16)
        value = work.tile([128, NSC, D], BF16)  # value rows [s,d] per s-chunk
        rT_sb = work.tile([128, 4, 256], BF16)  # retrievedT [d, t]
        rsq = work.tile([128, 4, 256], BF16)
        out_sb = work.tile([128, 2, D], FP32)

        # ---- MLP1 (key): hkT[j, :] = gelu(Wk1^T @ xT + bk1) ----
        # One 2-bank PSUM tile per j; m-halves are separate accumulation
        # groups (one start per bank), one big gelu over both.
        for j in range(4):
            ps = ps_big.tile([128, 1024], FP32, tag="mlp")
            for m in range(2):
                for c in range(4):
                    nc.tensor.matmul(
                        ps[:, m * 512:(m + 1) * 512],
                        lhsT=wk1[:, c, j * 128:(j + 1) * 128],
                        rhs=xT(c, m * 512, (m + 1) * 512),
                        start=(c == 0),
                        stop=(c == 3),
                    )
            nc.scalar.activation(
                out=hkT[:, j, :], in_=ps,
                func=GELU, bias=bk1[:, j:j + 1], scale=1.0,
            )

        # ---- MLP1 (query): 4 j-groups packed into one 2-bank tile ----
        ps_q = ps_big.tile([128, 1024], FP32, tag="mlp")
        for j in range(4):
            reg = ps_q[:, j * 256:(j + 1) * 256]
            for c in range(4):
                nc.tensor.matmul(
                    reg,
                    lhsT=wq1[:, c, j * 128:(j + 1) * 128],
                    rhs=qxT[:, c, :],
                    start=(c == 0 and j % 2 == 0),  # one start per bank
                    stop=(c == 3 and j % 2 == 1),
                )
        for j in range(4):  # per-j gelu (per-partition bias differs per j)
            nc.scalar.activation(
                out=hqT[:, j, :],
                in_=ps_q[:, j * 256:(j + 1) * 256],
                func=GELU, bias=bq1[:, j:j + 1], scale=1.0,
            )

        # ---- key phase matmul + tanh (duplicated halves via doubled Wk2) ----
        ps_k = ps_big.tile([128, 1024], FP32, tag="mlp")
        for m in range(2):
            for j in range(4):
                nc.tensor.matmul(
                    ps_k[:, m * 512:(m + 1) * 512],
                    lhsT=wk2d[:, j, :],
                    rhs=hkT[:, j, m * 512:(m + 1) * 512],
                    start=(j == 0),
                    stop=(j == 3),
                )
        nc.scalar.activation(out=kph2, in_=ps_k, func=AF.Tanh,
                             bias=bk2d, scale=1.0)
        # ---- query phase matmul + tanh ----
        ps_p = ps_big.tile([128, 1024], FP32, tag="mlp")
        for j in range(4):
            nc.tensor.matmul(
                ps_p[:, 0:256],
                lhsT=wq2d[:, j, :],
                rhs=hqT[:, j, :],
                start=(j == 0),
                stop=(j == 3),
            )
        nc.scalar.activation(out=qph2, in_=ps_p[:, 0:256], func=AF.Tanh,
                             bias=bq2d, scale=1.0)

        # ---- |t| on the cos half (ACT Abs; in every table set) ----
        nc.scalar.activation(out=kph2[0:64, :], in_=kph2[0:64, :], func=AF.Abs)
        nc.scalar.activation(out=qph2[0:64, :], in_=qph2[0:64, :], func=AF.Abs)

        # ---- value rows: value[s, d] = x@Wv + bv ----
        # own PSUM slots (at-pool, idle this early) so value never contends
        # with the phase matmuls for mlp-pool slots (priority inversion).
        for sc in range(NSC):
            ps = ps_at.tile([128, 512], FP32, tag="at")
            for c in range(4):
                nc.tensor.matmul(
                    ps,
                    lhsT=xT(c, sc * 128, (sc + 1) * 128),
                    rhs=wv[:, c, :],
                    start=(c == 0),
                    stop=(c == 3),
                )
            nc.vector.tensor_add(out=value[:, sc, :], in0=ps, in1=bvb)

        # ---- cos/sin of phases (stacked halves: 0:64 cos, 64:128 sin) ----
        # ACT Sin domain is [-pi, pi]; with t in (-1,1):
        #   cos(pi t) = sin(pi/2 - pi |t|),  sin(pi t) = sin(pi t)
        # one Sin pass with per-partition scale (-pi / +pi) and bias (pi/2 / 0).
        nc.scalar.activation(out=KS, in_=kph2, func=AF.Sin,
                             bias=cosbias, scale=sinscale)
        nc.scalar.activation(out=QS, in_=qph2, func=AF.Sin,
                             bias=cosbias, scale=sinscale)

        # ---- scores + causal mask + retrievedT accumulation ----
        rt_ps = ps_rt.tile([128, 4, 256], FP32)
        for sc in range(NSC):
            at_ps = ps_at.tile([128, 256], FP32, tag="at")
            nc.tensor.matmul(
                at_ps,
                lhsT=KS[:, sc * 128:(sc + 1) * 128],
                rhs=QS,
                start=True,
                stop=True,
            )
            atm = atm_pool.tile([128, 256], BF16, tag="atm")
            nc.vector.tensor_mul(out=atm, in0=at_ps, in1=maskt[:, sc, :])
            # rt_ps spans 2 banks (dc 0,1 | dc 2,3): exactly one start per
            # bank (first matmul into it), one stop on each bank's last.
            for dc in range(NDC):
                nc.tensor.matmul(
                    rt_ps[:, dc, :],
                    lhsT=value[:, sc, dc * 128:(dc + 1) * 128],
                    rhs=atm,
                    start=(sc == 0 and dc in (0, 2)),
                    stop=(sc == NSC - 1 and dc in (1, 3)),
                )

        # ---- retrievedT -> SBUF (ACT, idle by now) + squares (DVE) ----
        for dc in range(NDC):
            nc.scalar.copy(out=rT_sb[:, dc, :], in_=rt_ps[:, dc, :])
        for dc in range(NDC):
            nc.vector.tensor_mul(out=rsq[:, dc, :], in0=rT_sb[:, dc, :],
                                 in1=rT_sb[:, dc, :])

        # ---- row stats: sums/sumsq share one bank (one group); row sums
        # reuse the at-pool bank (free by now) with their own group ----
        sums_ps = ps_at.tile([128, 4], FP32, tag="at")  # [sum s0, sum s1, sq s0, sq s1]
        row_ps = ps_at.tile([1, 256], FP32, tag="at")
        first = True
        n = 0
        for st in range(2):
            for src, col in ((rT_sb, st), (rsq, 2 + st)):
                for dc in range(NDC):
                    n += 1
                    nc.tensor.matmul(
                        sums_ps[:, col:col + 1],
                        lhsT=src[:, dc, st * 128:(st + 1) * 128],
                        rhs=ones,
                        start=first,
                        stop=(n == 16),
                    )
                    first = False
        for dc in range(NDC):
            nc.tensor.matmul(
                row_ps,
                lhsT=ones,
                rhs=rT_sb[:, dc, :],
                start=(dc == 0),
                stop=(dc == 3),
            )

        # negmu_row = -(row sums)/D  (bf16, feeds the rank-1 mean-fold matmul)
        negmu = small.tile([1, 256], BF16)
        nc.vector.tensor_scalar_mul(out=negmu, in0=row_ps, scalar1=-1.0 / D)

        # per-strip scale_t = 1/sqrt(var + eps*norm^2)
        mu = small.tile([128, 2], FP32)
        musq = small.tile([128, 2], FP32)
        var = small.tile([128, 2], FP32)
        scl = small.tile([128, 2], FP32)
        for st in range(2):
            nc.vector.tensor_scalar_mul(out=mu[:, st:st + 1],
                                        in0=sums_ps[:, st:st + 1],
                                        scalar1=1.0 / D)
            nc.vector.tensor_mul(out=musq[:, st:st + 1],
                                 in0=mu[:, st:st + 1], in1=mu[:, st:st + 1])
            nc.vector.scalar_tensor_tensor(
                out=var[:, st:st + 1],
                in0=sums_ps[:, 2 + st:3 + st],
                scalar=1.0 / D,
                in1=musq[:, st:st + 1],
                op0=ALU.mult,
                op1=ALU.subtract,
            )
        for st in range(2):
            nc.scalar.activation(out=scl[:, st:st + 1], in_=var[:, st:st + 1],
                                 func=AF.Sqrt, bias=epsn2[:, st:st + 1],
                                 scale=1.0)
            nc.vector.reciprocal(out=scl[:, st:st + 1], in_=scl[:, st:st + 1])

        # ---- output: out = scale * (rT^T @ Wg - mu*cw) + xplus ----
        for st in range(2):
            ps = ps_big.tile([128, 1024], FP32, tag="mlp")
            reg = ps[:, 0:512]
            for dc in range(NDC):
                nc.tensor.matmul(
                    reg,
                    lhsT=rT_sb[:, dc, st * 128:(st + 1) * 128],
                    rhs=wg[:, dc, :],
                    start=(dc == 0),
                    stop=False,
                )
            nc.tensor.matmul(
                reg,
                lhsT=negmu[:, st * 128:(st + 1) * 128],
                rhs=cw,
                start=False,
                stop=True,
            )
            nc.vector.scalar_tensor_tensor(
                out=out_sb[:, st, :],
                in0=reg,
                scalar=scl[:, st:st + 1],
                in1=xplus[:, st, :],
                op0=ALU.mult,
                op1=ALU.add,
            )
            nc.sync.dma_start(out=d_out[st], in_=out_sb[:, st, :])

    return nc


def _host_prepare(inputs):
    """Build the 8 per-core input maps (host-side numpy packing)."""
    import ml_dtypes

    bf16 = ml_dtypes.bfloat16
    f32 = np.float32

    x = np.asarray(inputs["x"], f32)
    Wk1 = np.asarray(inputs["Wk1"], f32)
    bk1 = np.asarray(inputs["bk1"], f32)
    Wk2 = np.asarray(inputs["Wk2"], f32)
    bk2 = np.asarray(inputs["bk2"], f32)
    Wq1 = np.asarray(inputs["Wq1"], f32)
    bq1 = np.asarray(inputs["bq1"], f32)
    Wq2 = np.asarray(inputs["Wq2"], f32)
    bq2 = np.asarray(inputs["bq2"], f32)
    Wv = np.asarray(inputs["Wv"], f32)
    bv = np.asarray(inputs["bv"], f32)
    ln_g = np.asarray(inputs["ln_g"], f32)
    ln_b = np.asarray(inputs["ln_b"], f32)
    Wo = np.asarray(inputs["Wo"], f32)
    bo = np.asarray(inputs["bo"], f32)

    Wg32 = ln_g[:, None] * Wo
    cw = Wg32.astype(bf16).astype(f32).sum(axis=0).astype(bf16).reshape(1, D)
    out_bias = (ln_b @ Wo + bo).astype(f32)

    def pack(w):  # [D_in, F] -> [128, 4, F]
        return w.reshape(4, 128, -1).transpose(1, 0, 2)

    wk2d = np.concatenate([Wk2, Wk2], axis=1)  # [512, 128]
    wq2d = np.concatenate([Wq2, Wq2], axis=1)

    def fill(width, table, parts, dt):
        buf = np.zeros((128, width), dt)
        for name, arr in parts.items():
            off, w = table[name]
            buf[:, off:off + w] = arr.reshape(128, w).astype(dt)
        return buf

    early_base = {"wk1": pack(Wk1), "wk2d": pack(wk2d)}
    mid_base = {"wq1": pack(Wq1), "wq2d": pack(wq2d), "wv": pack(Wv)}
    f32a_arr = fill(F32A_W, F32A, {
        "bk1": bk1.reshape(4, 128).T,
        "bq1": bq1.reshape(4, 128).T,
        "bk2d": np.concatenate([bk2, bk2]).reshape(128, 1),
        "bq2d": np.concatenate([bq2, bq2]).reshape(128, 1),
        "bvb": np.broadcast_to(bv, (128, D)),
    }, f32)
    wg_packed = pack(Wg32)

    in_maps = []
    for c in range(NCORES):
        b, i = divmod(c, 4)
        t0, t1 = i * 128, (7 - i) * 128
        xb = x[b]  # [L, D]
        xTp = pack(np.ascontiguousarray(xb.T))  # [128, 4, L]
        qx = np.concatenate(
            [xb[t0:t0 + 128].T, xb[t1:t1 + 128].T], axis=1)  # [512, 256]
        tglob = np.concatenate(
            [np.arange(t0, t0 + 128), np.arange(t1, t1 + 128)])
        mask = (np.arange(L)[:, None] <= tglob[None, :])  # [L, 256]
        xplus = np.stack([xb[t0:t0 + 128], xb[t1:t1 + 128]]) + out_bias
        epsn2 = (EPS * K * (tglob.astype(f32) + 1.0)).reshape(2, 128)

        m = {
            "early_bf": fill(EARLY_BF_W, EARLY_BF,
                             {**early_base, "xTa": xTp[:, :, 0:512],
                              "xTb": xTp[:, :, 512:1024]}, bf16),
            "mid_bf": fill(MID_BF_W, MID_BF,
                           {**mid_base, "qxT": pack(qx)}, bf16),
            "f32a": f32a_arr,
            "late_bf": fill(LATE_BF_W, LATE_BF, {
                "mask": mask.reshape(NSC, 128, 256).transpose(1, 0, 2),
                "wg": wg_packed,
            }, bf16),
            "f32b": fill(F32B_W, F32B, {
                "xplus": xplus.transpose(1, 0, 2),
                "epsn2": epsn2.T,
            }, f32),
            "cw": cw,
        }
        in_maps.append(m)
    return in_maps


def run(inputs, trace=False):
    from concourse.bass_utils import run_bass_kernel_spmd

    if "nc" not in _CACHE:
        nc = _build_program()
        nc.finalize()
        _CACHE["nc"] = nc
    nc = _CACHE["nc"]
    in_maps = _host_prepare(inputs)
    res = run_bass_kernel_spmd(nc, in_maps, list(range(NCORES)), trace=trace)
    out = np.empty((B, L, D), np.float32)
    for c in range(NCORES):
        b, i = divmod(c, 4)
        oc = np.asarray(res.results[c]["out"], np.float32)
        out[b, i * 128:(i + 1) * 128] = oc[0]
        out[b, (7 - i) * 128:(8 - i) * 128] = oc[1]
    return out, res


def kernel(**inputs):
    out, _ = run(inputs, trace=False)
    return out


# revision 32
# speedup vs baseline: 1.1807x; 1.0501x over previous
"""Trainium2 Bass kernel for nn_BaselinePhasorBlock (B=2, L=1024, D=512, K=64).

Algorithm restructure: the phasor-memory cumsum
    retrieved[t,d] = Re[ sum_k e^{-i q[t,k]} * sum_{s<=t} e^{i key[s,k]} v[s,d] ]
collapses to causal attention:
    A[t,s] = cosQ[t]·cosK[s] + sinQ[t]·sinK[s]   (dot over k)
    retrieved = tril(A) @ value
so nothing of size (L,K,D) is ever materialized.

LayerNorm folding (exact):
    LN(retrieved/norm) @ Wo + bo + x
  = scale_t * (r @ Wg - mu_t * cw) + [x + ln_b@Wo + bo]
with Wg = diag(ln_g)@Wo, cw = colsums(Wg), scale_t = 1/sqrt(var_r + eps*norm_t^2),
norm_t^2 = (t+1)*K.  (LN row stats are scale-invariant up to the eps term, which
is folded into the eps exactly.)

Sharding (8 cores, SPMD, no collectives): core c -> batch b = c//4, strip pair
i = c%4 owning t-strips [i*128, (i+1)*128) and [(7-i)*128, (8-i)*128).  The
pairing makes causal work uniform; causality is enforced by a per-core mask on
the score matrix (AT layout [s, t]).  Each core computes its batch's keys and
values over the full sequence (redundant across the 4 cores of a batch, but
avoids collectives entirely).

All matmuls run in bf16 (validated: ~1.7e-3 scale-relative error end-to-end
vs the fp32 jax reference), fp32 PSUM accumulation, fp32 residual/output path.

Inputs are host-packed into a handful of layout-exact mega-tensors so the
kernel issues only ~6 DMAs, ordered by first use (DMA-issue serialization on
the sync sequencer was the dominant stall in v1).
"""

import math
from contextlib import ExitStack

import numpy as np

B, L, D, K = 2, 1024, 512, 64
PI = math.pi
NCORES = 8
NSC = L // 128  # 8 s-chunks
NDC = D // 128  # 4 d-chunks
EPS = 1e-5

# flat offsets (in elements) inside the packed DMA groups
EARLY_BF = {"wk1": (0, 2048), "xTa": (2048, 2048)}
EARLY_BF_W = 4096
EARLY2_BF = {"xTb": (0, 2048), "wk2d": (2048, 512)}
EARLY2_BF_W = 2560
MID_BF = {"wq1": (0, 2048), "qxT": (2048, 1024),
          "wq2d": (3072, 512), "wv": (3584, 2048)}
MID_BF_W = 5632
F32A = {"bk1": (0, 4), "bq1": (4, 4), "bk2d": (8, 1), "bq2d": (9, 1),
        "bvb": (10, 512)}
F32A_W = 522
LATE_BF = {"mask": (0, 2048), "wg": (2048, 2048)}
LATE_BF_W = 4096
F32B = {"xplus": (0, 1024), "epsn2": (1024, 2)}
F32B_W = 1026

_CACHE = {}


def _build_program(gelu_override=None):
    import concourse.bacc as bacc
    import concourse.mybir as mybir
    import concourse.tile as tile

    AF = mybir.ActivationFunctionType
    ALU = mybir.AluOpType
    GELU = AF.Gelu if gelu_override is None else gelu_override
    FP32 = mybir.dt.float32
    BF16 = mybir.dt.bfloat16

    nc = bacc.Bacc()

    d_early = nc.declare_dram_parameter("early_bf", [128, EARLY_BF_W], BF16, False)
    d_early2 = nc.declare_dram_parameter("early2_bf", [128, EARLY2_BF_W], BF16, False)
    d_f32a = nc.declare_dram_parameter("f32a", [128, F32A_W], FP32, False)
    d_mid = nc.declare_dram_parameter("mid_bf", [128, MID_BF_W], BF16, False)
    d_late = nc.declare_dram_parameter("late_bf", [128, LATE_BF_W], BF16, False)
    d_f32b = nc.declare_dram_parameter("f32b", [128, F32B_W], FP32, False)
    d_cw = nc.declare_dram_parameter("cw", [1, D], BF16, False)
    d_out = nc.declare_dram_parameter("out", [2, 128, D], FP32, True)

    with tile.TileContext(nc) as tc, ExitStack() as ctx:
        consts = ctx.enter_context(tc.tile_pool(name="consts", bufs=1))
        work = ctx.enter_context(tc.tile_pool(name="work", bufs=1))
        atm_pool = ctx.enter_context(tc.tile_pool(name="atm", bufs=3))
        small = ctx.enter_context(tc.tile_pool(name="small", bufs=1))
        ps_big = ctx.enter_context(tc.tile_pool(name="ps_big", bufs=2, space="PSUM"))
        ps_at = ctx.enter_context(tc.tile_pool(name="ps_at", bufs=2, space="PSUM"))
        ps_rt = ctx.enter_context(tc.tile_pool(name="ps_rt", bufs=1, space="PSUM"))

        # ---- SBUF tiles for the packed input groups + views ----
        early = consts.tile([128, EARLY_BF_W], BF16)
        early2 = consts.tile([128, EARLY2_BF_W], BF16)
        mid = consts.tile([128, MID_BF_W], BF16)
        f32a = consts.tile([128, F32A_W], FP32)
        late = consts.tile([128, LATE_BF_W], BF16)
        f32b = consts.tile([128, F32B_W], FP32)
        cw = consts.tile([1, D], BF16)
        ones = consts.tile([128, 1], BF16)
        cosbias = consts.tile([128, 1], FP32)
        sinscale = consts.tile([128, 1], FP32)

        def view(tile_, table, name, c=None):
            off, w = table[name]
            v = tile_[:, off:off + w]
            if c is not None:
                v = v.rearrange("p (c f) -> p c f", c=c)
            return v

        wk1 = view(early, EARLY_BF, "wk1", 4)     # [128, 4, 512]
        wk2d = view(early2, EARLY2_BF, "wk2d", 4)  # [128, 4, 128]
        xTa = view(early, EARLY_BF, "xTa", 4)     # [128, 4, 512]
        xTb = view(early2, EARLY2_BF, "xTb", 4)
        wq1 = view(mid, MID_BF, "wq1", 4)
        qxT = view(mid, MID_BF, "qxT", 4)         # [128, 4, 256]
        wq2d = view(mid, MID_BF, "wq2d", 4)
        wv = view(mid, MID_BF, "wv", 4)
        bk1 = view(f32a, F32A, "bk1")             # [128, 4]
        bq1 = view(f32a, F32A, "bq1")
        bk2d = view(f32a, F32A, "bk2d")           # [128, 1]
        bq2d = view(f32a, F32A, "bq2d")
        bvb = view(f32a, F32A, "bvb")             # [128, 512]
        maskt = view(late, LATE_BF, "mask", NSC)  # [128, 8, 256]
        wg = view(late, LATE_BF, "wg", 4)
        xplus = view(f32b, F32B, "xplus", 2)      # [128, 2, 512]
        epsn2 = view(f32b, F32B, "epsn2")         # [128, 2]

        def xT(c, lo, hi):
            """x^T slice [din-chunk c, seq cols lo:hi] across the a/b halves."""
            if hi <= 512:
                return xTa[:, c, lo:hi]
            return xTb[:, c, lo - 512:hi - 512]

        # ---- DMAs: one per group, issued in need-order on sync ----
        nc.sync.dma_start(out=early, in_=d_early[:])
        nc.sync.dma_start(out=f32a, in_=d_f32a[:])
        nc.sync.dma_start(out=early2, in_=d_early2[:])
        nc.sync.dma_start(out=mid, in_=d_mid[:])
        nc.sync.dma_start(out=late, in_=d_late[:])
        nc.sync.dma_start(out=f32b, in_=d_f32b[:])
        nc.sync.dma_start(out=cw, in_=d_cw[:])
        nc.vector.memset(ones, 1.0)
        nc.vector.memset(cosbias[0:64, :], PI / 2)
        nc.vector.memset(cosbias[64:128, :], 0.0)
        nc.vector.memset(sinscale[0:64, :], -PI)
        nc.vector.memset(sinscale[64:128, :], PI)

        # ---- working SBUF tiles ----
        hkT = work.tile([128, 4, L], BF16)      # gelu(x@Wk1+b) transposed
        hqT = work.tile([128, 4, 256], BF16)
        kph2 = work.tile([128, L], BF16)        # tanh phase, duplicated halves
        qph2 = work.tile([128, 256], BF16)
        KS = work.tile([128, L], BF16)          # rows 0:64 cosK, 64:128 sinK
        QS = work.tile([128, 256], BF16)
        value = work.tile([128, NSC, D], BF16)  # value rows [s,d] per s-chunk
        rT_sb = work.tile([128, 4, 256], BF16)  # retrievedT [d, t]
        rsq = work.tile([128, 4, 256], BF16)
        out_sb = work.tile([128, 2, D], FP32)

        # ---- MLP1 (key): hkT[j, m] = gelu(Wk1^T @ xT + bk1) ----
        # per-(j,m) 1-bank groups; all m0 first (needs only the first DMA)
        for m in range(2):
            for j in range(4):
                ps = ps_big.tile([128, 512], FP32, tag="mlp")
                for c in range(4):
                    nc.tensor.matmul(
                        ps,
                        lhsT=wk1[:, c, j * 128:(j + 1) * 128],
                        rhs=xT(c, m * 512, (m + 1) * 512),
                        start=(c == 0),
                        stop=(c == 3),
                    )
                nc.scalar.activation(
                    out=hkT[:, j, m * 512:(m + 1) * 512], in_=ps,
                    func=GELU, bias=bk1[:, j:j + 1], scale=1.0,
                )

        # ---- MLP1 (query): per-j groups ----
        for j in range(4):
            ps = ps_big.tile([128, 512], FP32, tag="mlp")
            for c in range(4):
                nc.tensor.matmul(
                    ps[:, 0:256],
                    lhsT=wq1[:, c, j * 128:(j + 1) * 128],
                    rhs=qxT[:, c, :],
                    start=(c == 0),
                    stop=(c == 3),
                )
            nc.scalar.activation(
                out=hqT[:, j, :],
                in_=ps[:, 0:256],
                func=GELU, bias=bq1[:, j:j + 1], scale=1.0,
            )

        # ---- key phase matmul + tanh (duplicated halves via doubled Wk2) ----
        for m in range(2):
            ps_k = ps_big.tile([128, 512], FP32, tag="mlp")
            for j in range(4):
                nc.tensor.matmul(
                    ps_k,
                    lhsT=wk2d[:, j, :],
                    rhs=hkT[:, j, m * 512:(m + 1) * 512],
                    start=(j == 0),
                    stop=(j == 3),
                )
            nc.scalar.activation(out=kph2[:, m * 512:(m + 1) * 512],
                                 in_=ps_k, func=AF.Tanh, bias=bk2d, scale=1.0)
        # ---- query phase matmul + tanh ----
        ps_p = ps_big.tile([128, 512], FP32, tag="mlp")
        for j in range(4):
            nc.tensor.matmul(
                ps_p[:, 0:256],
                lhsT=wq2d[:, j, :],
                rhs=hqT[:, j, :],
                start=(j == 0),
                stop=(j == 3),
            )
        nc.scalar.activation(out=qph2, in_=ps_p[:, 0:256], func=AF.Tanh,
                             bias=bq2d, scale=1.0)

        # ---- |t| on the cos half (ACT Abs; in every table set) ----
        nc.scalar.activation(out=kph2[0:64, :], in_=kph2[0:64, :], func=AF.Abs)
        nc.scalar.activation(out=qph2[0:64, :], in_=qph2[0:64, :], func=AF.Abs)

        # ---- value rows: value[s, d] = x@Wv + bv ----
        # own PSUM slots (at-pool, idle this early) so value never contends
        # with the phase matmuls for mlp-pool slots (priority inversion).
        for sc in range(NSC):
            ps = ps_at.tile([128, 512], FP32, tag="at")
            for c in range(4):
                nc.tensor.matmul(
                    ps,
                    lhsT=xT(c, sc * 128, (sc + 1) * 128),
                    rhs=wv[:, c, :],
                    start=(c == 0),
                    stop=(c == 3),
                )
            nc.vector.tensor_add(out=value[:, sc, :], in0=ps, in1=bvb)

        # ---- cos/sin of phases (stacked halves: 0:64 cos, 64:128 sin) ----
        # ACT Sin domain is [-pi, pi]; with t in (-1,1):
        #   cos(pi t) = sin(pi/2 - pi |t|),  sin(pi t) = sin(pi t)
        # one Sin pass with per-partition scale (-pi / +pi) and bias (pi/2 / 0).
        nc.scalar.activation(out=KS, in_=kph2, func=AF.Sin,
                             bias=cosbias, scale=sinscale)
        nc.scalar.activation(out=QS, in_=qph2, func=AF.Sin,
                             bias=cosbias, scale=sinscale)

        # ---- scores + causal mask + retrievedT accumulation ----
        rt_ps = ps_rt.tile([128, 4, 256], FP32)
        for sc in range(NSC):
            at_ps = ps_at.tile([128, 256], FP32, tag="at")
            nc.tensor.matmul(
                at_ps,
                lhsT=KS[:, sc * 128:(sc + 1) * 128],
                rhs=QS,
                start=True,
                stop=True,
            )
            atm = atm_pool.tile([128, 256], BF16, tag="atm")
            nc.vector.tensor_mul(out=atm, in0=at_ps, in1=maskt[:, sc, :])
            # rt_ps spans 2 banks (dc 0,1 | dc 2,3): exactly one start per
            # bank (first matmul into it), one stop on each bank's last.
            for dc in range(NDC):
                nc.tensor.matmul(
                    rt_ps[:, dc, :],
                    lhsT=value[:, sc, dc * 128:(dc + 1) * 128],
                    rhs=atm,
                    start=(sc == 0 and dc in (0, 2)),
                    stop=(sc == NSC - 1 and dc in (1, 3)),
                )

        # ---- retrievedT -> SBUF (ACT, idle by now) + squares (DVE) ----
        for dc in range(NDC):
            nc.scalar.copy(out=rT_sb[:, dc, :], in_=rt_ps[:, dc, :])
        for dc in range(NDC):
            nc.vector.tensor_mul(out=rsq[:, dc, :], in0=rT_sb[:, dc, :],
                                 in1=rT_sb[:, dc, :])

        # ---- row stats: sums/sumsq share one bank (one group); row sums
        # reuse the at-pool bank (free by now) with their own group ----
        sums_ps = ps_at.tile([128, 4], FP32, tag="at")  # [sum s0, sum s1, sq s0, sq s1]
        row_ps = ps_at.tile([1, 256], FP32, tag="at")
        first = True
        n = 0
        for st in range(2):
            for src, col in ((rT_sb, st), (rsq, 2 + st)):
                for dc in range(NDC):
                    n += 1
                    nc.tensor.matmul(
                        sums_ps[:, col:col + 1],
                        lhsT=src[:, dc, st * 128:(st + 1) * 128],
                        rhs=ones,
                        start=first,
                        stop=(n == 16),
                    )
                    first = False
        for dc in range(NDC):
            nc.tensor.matmul(
                row_ps,
                lhsT=ones,
                rhs=rT_sb[:, dc, :],
                start=(dc == 0),
                stop=(dc == 3),
            )

        # negmu_row = -(row sums)/D  (bf16, feeds the rank-1 mean-fold matmul)
        negmu = small.tile([1, 256], BF16)
        nc.vector.tensor_scalar_mul(out=negmu, in0=row_ps, scalar1=-1.0 / D)

        # per-strip scale_t = 1/sqrt(var + eps*norm^2)
        mu = small.tile([128, 2], FP32)
        musq = small.tile([128, 2], FP32)
        var = small.tile([128, 2], FP32)
        scl = small.tile([128, 2], FP32)
        for st in range(2):
            nc.vector.tensor_scalar_mul(out=mu[:, st:st + 1],
                                        in0=sums_ps[:, st:st + 1],
                                        scalar1=1.0 / D)
            nc.vector.tensor_mul(out=musq[:, st:st + 1],
                                 in0=mu[:, st:st + 1], in1=mu[:, st:st + 1])
            nc.vector.scalar_tensor_tensor(
                out=var[:, st:st + 1],
                in0=sums_ps[:, 2 + st:3 + st],
                scalar=1.0 / D,
                in1=musq[:, st:st + 1],
                op0=ALU.mult,
                op1=ALU.subtract,
            )
        for st in range(2):
            nc.scalar.activation(out=scl[:, st:st + 1], in_=var[:, st:st + 1],
                                 func=AF.Sqrt, bias=epsn2[:, st:st + 1],
                                 scale=1.0)
            nc.vector.reciprocal(out=scl[:, st:st + 1], in_=scl[:, st:st + 1])

        # ---- output: out = scale * (rT^T @ Wg - mu*cw) + xplus ----
        for st in range(2):
            ps = ps_big.tile([128, 512], FP32, tag="mlp")
            reg = ps
            for dc in range(NDC):
                nc.tensor.matmul(
                    reg,
                    lhsT=rT_sb[:, dc, st * 128:(st + 1) * 128],
                    rhs=wg[:, dc, :],
                    start=(dc == 0),
                    stop=False,
                )
            nc.tensor.matmul(
                reg,
                lhsT=negmu[:, st * 128:(st + 1) * 128],
                rhs=cw,
                start=False,
                stop=True,
            )
            nc.vector.scalar_tensor_tensor(
                out=out_sb[:, st, :],
                in0=reg,
                scalar=scl[:, st:st + 1],
                in1=xplus[:, st, :],
                op0=ALU.mult,
                op1=ALU.add,
            )
            nc.sync.dma_start(out=d_out[st], in_=out_sb[:, st, :])

    return nc


def _host_prepare(inputs):
    """Build the 8 per-core input maps (host-side numpy packing)."""
    import ml_dtypes

    bf16 = ml_dtypes.bfloat16
    f32 = np.float32

    x = np.asarray(inputs["x"], f32)
    Wk1 = np.asarray(inputs["Wk1"], f32)
    bk1 = np.asarray(inputs["bk1"], f32)
    Wk2 = np.asarray(inputs["Wk2"], f32)
    bk2 = np.asarray(inputs["bk2"], f32)
    Wq1 = np.asarray(inputs["Wq1"], f32)
    bq1 = np.asarray(inputs["bq1"], f32)
    Wq2 = np.asarray(inputs["Wq2"], f32)
    bq2 = np.asarray(inputs["bq2"], f32)
    Wv = np.asarray(inputs["Wv"], f32)
    bv = np.asarray(inputs["bv"], f32)
    ln_g = np.asarray(inputs["ln_g"], f32)
    ln_b = np.asarray(inputs["ln_b"], f32)
    Wo = np.asarray(inputs["Wo"], f32)
    bo = np.asarray(inputs["bo"], f32)

    Wg32 = ln_g[:, None] * Wo
    cw = Wg32.astype(bf16).astype(f32).sum(axis=0).astype(bf16).reshape(1, D)
    out_bias = (ln_b @ Wo + bo).astype(f32)

    def pack(w):  # [D_in, F] -> [128, 4, F]
        return w.reshape(4, 128, -1).transpose(1, 0, 2)

    wk2d = np.concatenate([Wk2, Wk2], axis=1)  # [512, 128]
    wq2d = np.concatenate([Wq2, Wq2], axis=1)

    def fill(width, table, parts, dt):
        buf = np.zeros((128, width), dt)
        for name, arr in parts.items():
            off, w = table[name]
            buf[:, off:off + w] = arr.reshape(128, w).astype(dt)
        return buf

    early_base = {"wk1": pack(Wk1)}
    early2_base = {"wk2d": pack(wk2d)}
    mid_base = {"wq1": pack(Wq1), "wq2d": pack(wq2d), "wv": pack(Wv)}
    f32a_arr = fill(F32A_W, F32A, {
        "bk1": bk1.reshape(4, 128).T,
        "bq1": bq1.reshape(4, 128).T,
        "bk2d": np.concatenate([bk2, bk2]).reshape(128, 1),
        "bq2d": np.concatenate([bq2, bq2]).reshape(128, 1),
        "bvb": np.broadcast_to(bv, (128, D)),
    }, f32)
    wg_packed = pack(Wg32)

    in_maps = []
    for c in range(NCORES):
        b, i = divmod(c, 4)
        t0, t1 = i * 128, (7 - i) * 128
        xb = x[b]  # [L, D]
        xTp = pack(np.ascontiguousarray(xb.T))  # [128, 4, L]
        qx = np.concatenate(
            [xb[t0:t0 + 128].T, xb[t1:t1 + 128].T], axis=1)  # [512, 256]
        tglob = np.concatenate(
            [np.arange(t0, t0 + 128), np.arange(t1, t1 + 128)])
        mask = (np.arange(L)[:, None] <= tglob[None, :])  # [L, 256]
        xplus = np.stack([xb[t0:t0 + 128], xb[t1:t1 + 128]]) + out_bias
        epsn2 = (EPS * K * (tglob.astype(f32) + 1.0)).reshape(2, 128)

        m = {
            "early_bf": fill(EARLY_BF_W, EARLY_BF,
                             {**early_base, "xTa": xTp[:, :, 0:512]}, bf16),
            "early2_bf": fill(EARLY2_BF_W, EARLY2_BF,
                              {**early2_base,
                               "xTb": xTp[:, :, 512:1024]}, bf16),
            "mid_bf": fill(MID_BF_W, MID_BF,
                           {**mid_base, "qxT": pack(qx)}, bf16),
            "f32a": f32a_arr,
            "late_bf": fill(LATE_BF_W, LATE_BF, {
                "mask": mask.reshape(NSC, 128, 256).transpose(1, 0, 2),
                "wg": wg_packed,
            }, bf16),
            "f32b": fill(F32B_W, F32B, {
                "xplus": xplus.transpose(1, 0, 2),
                "epsn2": epsn2.T,
            }, f32),
            "cw": cw,
        }
        in_maps.append(m)
    return in_maps


def run(inputs, trace=False):
    from concourse.bass_utils import run_bass_kernel_spmd

    if "nc" not in _CACHE:
        nc = _build_program()
        nc.finalize()
        _CACHE["nc"] = nc
    nc = _CACHE["nc"]
    in_maps = _host_prepare(inputs)
    res = run_bass_kernel_spmd(nc, in_maps, list(range(NCORES)), trace=trace)
    out = np.empty((B, L, D), np.float32)
    for c in range(NCORES):
        b, i = divmod(c, 4)
        oc = np.asarray(res.results[c]["out"], np.float32)
        out[b, i * 128:(i + 1) * 128] = oc[0]
        out[b, (7 - i) * 128:(8 - i) * 128] = oc[1]
    return out, res


def kernel(**inputs):
    out, _ = run(inputs, trace=False)
    return out


# revision 35
# speedup vs baseline: 1.2025x; 1.0185x over previous
"""Trainium2 Bass kernel for nn_BaselinePhasorBlock (B=2, L=1024, D=512, K=64).

Algorithm restructure: the phasor-memory cumsum
    retrieved[t,d] = Re[ sum_k e^{-i q[t,k]} * sum_{s<=t} e^{i key[s,k]} v[s,d] ]
collapses to causal attention:
    A[t,s] = cosQ[t]·cosK[s] + sinQ[t]·sinK[s]   (dot over k)
    retrieved = tril(A) @ value
so nothing of size (L,K,D) is ever materialized.

LayerNorm folding (exact):
    LN(retrieved/norm) @ Wo + bo + x
  = scale_t * (r @ Wg - mu_t * cw) + [x + ln_b@Wo + bo]
with Wg = diag(ln_g)@Wo, cw = colsums(Wg), scale_t = 1/sqrt(var_r + eps*norm_t^2),
norm_t^2 = (t+1)*K.  (LN row stats are scale-invariant up to the eps term, which
is folded into the eps exactly.)

Sharding (8 cores, SPMD, no collectives): core c -> batch b = c//4, strip pair
i = c%4 owning t-strips [i*128, (i+1)*128) and [(7-i)*128, (8-i)*128).  The
pairing makes causal work uniform; causality is enforced by a per-core mask on
the score matrix (AT layout [s, t]).  Each core computes its batch's keys and
values over the full sequence (redundant across the 4 cores of a batch, but
avoids collectives entirely).

All matmuls run in bf16 (validated: ~1.7e-3 scale-relative error end-to-end
vs the fp32 jax reference), fp32 PSUM accumulation, fp32 residual/output path.

Inputs are host-packed into a handful of layout-exact mega-tensors so the
kernel issues only ~6 DMAs, ordered by first use (DMA-issue serialization on
the sync sequencer was the dominant stall in v1).
"""

import math
from contextlib import ExitStack

import numpy as np

B, L, D, K = 2, 1024, 512, 64
PI = math.pi
NCORES = 8
NSC = L // 128  # 8 s-chunks
NDC = D // 128  # 4 d-chunks
EPS = 1e-5

# flat offsets (in elements) inside the packed DMA groups
EARLY_BF = {"wk1": (0, 2048), "xTa": (2048, 2048)}
EARLY_BF_W = 4096
EARLY2_BF = {"xTb": (0, 2048), "wk2d": (2048, 512)}
EARLY2_BF_W = 2560
MID_BF = {"wq1": (0, 2048), "qxT": (2048, 1024),
          "wq2d": (3072, 512), "wv": (3584, 2048)}
MID_BF_W = 5632
F32A = {"bk1": (0, 4), "bq1": (4, 4), "bk2d": (8, 1), "bq2d": (9, 1),
        "bvb": (10, 512)}
F32A_W = 522
LATE_BF = {"mask": (0, 2048), "wg": (2048, 2048)}
LATE_BF_W = 4096
F32B = {"xplus": (0, 1024), "epsn2": (1024, 2)}
F32B_W = 1026

_CACHE = {}


def _build_program(gelu_override=None):
    import concourse.bacc as bacc
    import concourse.mybir as mybir
    import concourse.tile as tile

    AF = mybir.ActivationFunctionType
    ALU = mybir.AluOpType
    GELU = AF.Gelu if gelu_override is None else gelu_override
    FP32 = mybir.dt.float32
    BF16 = mybir.dt.bfloat16

    nc = bacc.Bacc()

    d_early = nc.declare_dram_parameter("early_bf", [128, EARLY_BF_W], BF16, False)
    d_early2 = nc.declare_dram_parameter("early2_bf", [128, EARLY2_BF_W], BF16, False)
    d_f32a = nc.declare_dram_parameter("f32a", [128, F32A_W], FP32, False)
    d_mid = nc.declare_dram_parameter("mid_bf", [128, MID_BF_W], BF16, False)
    d_late = nc.declare_dram_parameter("late_bf", [128, LATE_BF_W], BF16, False)
    d_f32b = nc.declare_dram_parameter("f32b", [128, F32B_W], FP32, False)
    d_cw = nc.declare_dram_parameter("cw", [1, D], BF16, False)
    d_out = nc.declare_dram_parameter("out", [2, 128, D], FP32, True)

    with tile.TileContext(nc) as tc, ExitStack() as ctx:
        consts = ctx.enter_context(tc.tile_pool(name="consts", bufs=1))
        work = ctx.enter_context(tc.tile_pool(name="work", bufs=1))
        atm_pool = ctx.enter_context(tc.tile_pool(name="atm", bufs=4))
        small = ctx.enter_context(tc.tile_pool(name="small", bufs=1))
        ps_big = ctx.enter_context(tc.tile_pool(name="ps_big", bufs=2, space="PSUM"))
        ps_at = ctx.enter_context(tc.tile_pool(name="ps_at", bufs=3, space="PSUM"))
        ps_rt = ctx.enter_context(tc.tile_pool(name="ps_rt", bufs=1, space="PSUM"))

        # ---- SBUF tiles for the packed input groups + views ----
        early = consts.tile([128, EARLY_BF_W], BF16)
        early2 = consts.tile([128, EARLY2_BF_W], BF16)
        mid = consts.tile([128, MID_BF_W], BF16)
        f32a = consts.tile([128, F32A_W], FP32)
        late = consts.tile([128, LATE_BF_W], BF16)
        f32b = consts.tile([128, F32B_W], FP32)
        cw = consts.tile([1, D], BF16)
        ones = consts.tile([128, 1], BF16)
        cosbias = consts.tile([128, 1], FP32)
        sinscale = consts.tile([128, 1], FP32)

        def view(tile_, table, name, c=None):
            off, w = table[name]
            v = tile_[:, off:off + w]
            if c is not None:
                v = v.rearrange("p (c f) -> p c f", c=c)
            return v

        wk1 = view(early, EARLY_BF, "wk1", 4)     # [128, 4, 512]
        wk2d = view(early2, EARLY2_BF, "wk2d", 4)  # [128, 4, 128]
        xTa = view(early, EARLY_BF, "xTa", 4)     # [128, 4, 512]
        xTb = view(early2, EARLY2_BF, "xTb", 4)
        wq1 = view(mid, MID_BF, "wq1", 4)
        qxT = view(mid, MID_BF, "qxT", 4)         # [128, 4, 256]
        wq2d = view(mid, MID_BF, "wq2d", 4)
        wv = view(mid, MID_BF, "wv", 4)
        bk1 = view(f32a, F32A, "bk1")             # [128, 4]
        bq1 = view(f32a, F32A, "bq1")
        bk2d = view(f32a, F32A, "bk2d")           # [128, 1]
        bq2d = view(f32a, F32A, "bq2d")
        bvb = view(f32a, F32A, "bvb")             # [128, 512]
        maskt = view(late, LATE_BF, "mask", NSC)  # [128, 8, 256]
        wg = view(late, LATE_BF, "wg", 4)
        xplus = view(f32b, F32B, "xplus", 2)      # [128, 2, 512]
        epsn2 = view(f32b, F32B, "epsn2")         # [128, 2]

        def xT(c, lo, hi):
            """x^T slice [din-chunk c, seq cols lo:hi] across the a/b halves."""
            if hi <= 512:
                return xTa[:, c, lo:hi]
            return xTb[:, c, lo - 512:hi - 512]

        # ---- DMAs: one per group, issued in need-order on sync ----
        nc.sync.dma_start(out=early, in_=d_early[:])
        nc.sync.dma_start(out=f32a, in_=d_f32a[:])
        nc.sync.dma_start(out=early2, in_=d_early2[:])
        nc.sync.dma_start(out=mid, in_=d_mid[:])
        nc.sync.dma_start(out=late, in_=d_late[:])
        nc.sync.dma_start(out=f32b, in_=d_f32b[:])
        nc.sync.dma_start(out=cw, in_=d_cw[:])
        nc.vector.memset(ones, 1.0)
        nc.vector.memset(cosbias[0:64, :], PI / 2)
        nc.vector.memset(cosbias[64:128, :], 0.0)
        nc.vector.memset(sinscale[0:64, :], -PI)
        nc.vector.memset(sinscale[64:128, :], PI)

        # ---- working SBUF tiles ----
        hkT = work.tile([128, 4, L], BF16)      # gelu(x@Wk1+b) transposed
        hqT = work.tile([128, 4, 256], BF16)
        kph2 = work.tile([128, L], BF16)        # tanh phase, duplicated halves
        qph2 = work.tile([128, 256], BF16)
        KS = work.tile([128, L], BF16)          # rows 0:64 cosK, 64:128 sinK
        QS = work.tile([128, 256], BF16)
        value = work.tile([128, NSC, D], BF16)  # value rows [s,d] per s-chunk
        rT_sb = work.tile([128, 4, 256], BF16)  # retrievedT [d, t]
        rsq = work.tile([128, 4, 256], BF16)
        out_sb = work.tile([128, 2, D], FP32)

        # ---- MLP1 (key): hkT[j, m] = gelu(Wk1^T @ xT + bk1) ----
        # per-(j,m) 1-bank groups; all m0 first (needs only the first DMA)
        for m in range(2):
            for j in range(4):
                ps = ps_big.tile([128, 512], FP32, tag="mlp")
                for c in range(4):
                    nc.tensor.matmul(
                        ps,
                        lhsT=wk1[:, c, j * 128:(j + 1) * 128],
                        rhs=xT(c, m * 512, (m + 1) * 512),
                        start=(c == 0),
                        stop=(c == 3),
                    )
                nc.scalar.activation(
                    out=hkT[:, j, m * 512:(m + 1) * 512], in_=ps,
                    func=GELU, bias=bk1[:, j:j + 1], scale=1.0,
                )

        # ---- MLP1 (query): per-j groups ----
        for j in range(4):
            ps = ps_big.tile([128, 512], FP32, tag="mlp")
            for c in range(4):
                nc.tensor.matmul(
                    ps[:, 0:256],
                    lhsT=wq1[:, c, j * 128:(j + 1) * 128],
                    rhs=qxT[:, c, :],
                    start=(c == 0),
                    stop=(c == 3),
                )
            nc.scalar.activation(
                out=hqT[:, j, :],
                in_=ps[:, 0:256],
                func=GELU, bias=bq1[:, j:j + 1], scale=1.0,
            )

        # ---- key phase matmul + tanh (duplicated halves via doubled Wk2) ----
        for m in range(2):
            ps_k = ps_big.tile([128, 512], FP32, tag="mlp")
            for j in range(4):
                nc.tensor.matmul(
                    ps_k,
                    lhsT=wk2d[:, j, :],
                    rhs=hkT[:, j, m * 512:(m + 1) * 512],
                    start=(j == 0),
                    stop=(j == 3),
                )
            nc.scalar.activation(out=kph2[:, m * 512:(m + 1) * 512],
                                 in_=ps_k, func=AF.Tanh, bias=bk2d, scale=1.0)
        # ---- query phase matmul + tanh ----
        ps_p = ps_big.tile([128, 512], FP32, tag="mlp")
        for j in range(4):
            nc.tensor.matmul(
                ps_p[:, 0:256],
                lhsT=wq2d[:, j, :],
                rhs=hqT[:, j, :],
                start=(j == 0),
                stop=(j == 3),
            )
        nc.scalar.activation(out=qph2, in_=ps_p[:, 0:256], func=AF.Tanh,
                             bias=bq2d, scale=1.0)

        # ---- |t| on the cos half (ACT Abs; in every table set) ----
        nc.scalar.activation(out=kph2[0:64, :], in_=kph2[0:64, :], func=AF.Abs)
        nc.scalar.activation(out=qph2[0:64, :], in_=qph2[0:64, :], func=AF.Abs)

        # ---- value rows: value[s, d] = x@Wv + bv ----
        # own PSUM slots (at-pool, idle this early) so value never contends
        # with the phase matmuls for mlp-pool slots (priority inversion).
        for sc in range(NSC):
            ps = ps_at.tile([128, 512], FP32, tag="at")
            for c in range(4):
                nc.tensor.matmul(
                    ps,
                    lhsT=xT(c, sc * 128, (sc + 1) * 128),
                    rhs=wv[:, c, :],
                    start=(c == 0),
                    stop=(c == 3),
                )
            nc.vector.tensor_add(out=value[:, sc, :], in0=ps, in1=bvb)

        # ---- cos/sin of phases (stacked halves: 0:64 cos, 64:128 sin) ----
        # ACT Sin domain is [-pi, pi]; with t in (-1,1):
        #   cos(pi t) = sin(pi/2 - pi |t|),  sin(pi t) = sin(pi t)
        # one Sin pass with per-partition scale (-pi / +pi) and bias (pi/2 / 0).
        nc.scalar.activation(out=KS, in_=kph2, func=AF.Sin,
                             bias=cosbias, scale=sinscale)
        nc.scalar.activation(out=QS, in_=qph2, func=AF.Sin,
                             bias=cosbias, scale=sinscale)

        # ---- scores + causal mask + retrievedT accumulation ----
        rt_ps = ps_rt.tile([128, 4, 256], FP32)
        for sc in range(NSC):
            at_ps = ps_at.tile([128, 256], FP32, tag="at")
            nc.tensor.matmul(
                at_ps,
                lhsT=KS[:, sc * 128:(sc + 1) * 128],
                rhs=QS,
                start=True,
                stop=True,
            )
            atm = atm_pool.tile([128, 256], BF16, tag="atm")
            nc.vector.tensor_mul(out=atm, in0=at_ps, in1=maskt[:, sc, :])
            # rt_ps spans 2 banks (dc 0,1 | dc 2,3): exactly one start per
            # bank (first matmul into it), one stop on each bank's last.
            for dc in range(NDC):
                nc.tensor.matmul(
                    rt_ps[:, dc, :],
                    lhsT=value[:, sc, dc * 128:(dc + 1) * 128],
                    rhs=atm,
                    start=(sc == 0 and dc in (0, 2)),
                    stop=(sc == NSC - 1 and dc in (1, 3)),
                )

        # ---- retrievedT -> SBUF (ACT, idle by now) + squares (DVE) ----
        for dc in range(NDC):
            nc.scalar.copy(out=rT_sb[:, dc, :], in_=rt_ps[:, dc, :])
        for dc in range(NDC):
            nc.vector.tensor_mul(out=rsq[:, dc, :], in0=rT_sb[:, dc, :],
                                 in1=rT_sb[:, dc, :])

        # ---- row stats: sums/sumsq share one bank (one group); row sums
        # reuse the at-pool bank (free by now) with their own group ----
        sums_ps = ps_at.tile([128, 4], FP32, tag="at")  # [sum s0, sum s1, sq s0, sq s1]
        row_ps = ps_at.tile([1, 256], FP32, tag="at")
        first = True
        n = 0
        for st in range(2):
            for src, col in ((rT_sb, st), (rsq, 2 + st)):
                for dc in range(NDC):
                    n += 1
                    nc.tensor.matmul(
                        sums_ps[:, col:col + 1],
                        lhsT=src[:, dc, st * 128:(st + 1) * 128],
                        rhs=ones,
                        start=first,
                        stop=(n == 16),
                    )
                    first = False
        for dc in range(NDC):
            nc.tensor.matmul(
                row_ps,
                lhsT=ones,
                rhs=rT_sb[:, dc, :],
                start=(dc == 0),
                stop=(dc == 3),
            )

        # negmu_row = -(row sums)/D  (bf16, feeds the rank-1 mean-fold matmul)
        negmu = small.tile([1, 256], BF16)
        nc.vector.tensor_scalar_mul(out=negmu, in0=row_ps, scalar1=-1.0 / D)

        # per-strip scale_t = 1/sqrt(var + eps*norm^2)
        mu = small.tile([128, 2], FP32)
        musq = small.tile([128, 2], FP32)
        var = small.tile([128, 2], FP32)
        scl = small.tile([128, 2], FP32)
        for st in range(2):
            nc.vector.tensor_scalar_mul(out=mu[:, st:st + 1],
                                        in0=sums_ps[:, st:st + 1],
                                        scalar1=1.0 / D)
            nc.vector.tensor_mul(out=musq[:, st:st + 1],
                                 in0=mu[:, st:st + 1], in1=mu[:, st:st + 1])
            nc.vector.scalar_tensor_tensor(
                out=var[:, st:st + 1],
                in0=sums_ps[:, 2 + st:3 + st],
                scalar=1.0 / D,
                in1=musq[:, st:st + 1],
                op0=ALU.mult,
                op1=ALU.subtract,
            )
        for st in range(2):
            nc.scalar.activation(out=scl[:, st:st + 1], in_=var[:, st:st + 1],
                                 func=AF.Sqrt, bias=epsn2[:, st:st + 1],
                                 scale=1.0)
            nc.vector.reciprocal(out=scl[:, st:st + 1], in_=scl[:, st:st + 1])

        # ---- output: out = scale * (rT^T @ Wg - mu*cw) + xplus ----
        for st in range(2):
            ps = ps_big.tile([128, 512], FP32, tag="mlp")
            reg = ps
            for dc in range(NDC):
                nc.tensor.matmul(
                    reg,
                    lhsT=rT_sb[:, dc, st * 128:(st + 1) * 128],
                    rhs=wg[:, dc, :],
                    start=(dc == 0),
                    stop=False,
                )
            nc.tensor.matmul(
                reg,
                lhsT=negmu[:, st * 128:(st + 1) * 128],
                rhs=cw,
                start=False,
                stop=True,
            )
            nc.vector.scalar_tensor_tensor(
                out=out_sb[:, st, :],
                in0=reg,
                scalar=scl[:, st:st + 1],
                in1=xplus[:, st, :],
                op0=ALU.mult,
                op1=ALU.add,
            )
            nc.sync.dma_start(out=d_out[st], in_=out_sb[:, st, :])

    return nc


def _host_prepare(inputs):
    """Build the 8 per-core input maps (host-side numpy packing)."""
    import ml_dtypes

    bf16 = ml_dtypes.bfloat16
    f32 = np.float32

    x = np.asarray(inputs["x"], f32)
    Wk1 = np.asarray(inputs["Wk1"], f32)
    bk1 = np.asarray(inputs["bk1"], f32)
    Wk2 = np.asarray(inputs["Wk2"], f32)
    bk2 = np.asarray(inputs["bk2"], f32)
    Wq1 = np.asarray(inputs["Wq1"], f32)
    bq1 = np.asarray(inputs["bq1"], f32)
    Wq2 = np.asarray(inputs["Wq2"], f32)
    bq2 = np.asarray(inputs["bq2"], f32)
    Wv = np.asarray(inputs["Wv"], f32)
    bv = np.asarray(inputs["bv"], f32)
    ln_g = np.asarray(inputs["ln_g"], f32)
    ln_b = np.asarray(inputs["ln_b"], f32)
    Wo = np.asarray(inputs["Wo"], f32)
    bo = np.asarray(inputs["bo"], f32)

    Wg32 = ln_g[:, None] * Wo
    cw = Wg32.astype(bf16).astype(f32).sum(axis=0).astype(bf16).reshape(1, D)
    out_bias = (ln_b @ Wo + bo).astype(f32)

    def pack(w):  # [D_in, F] -> [128, 4, F]
        return w.reshape(4, 128, -1).transpose(1, 0, 2)

    wk2d = np.concatenate([Wk2, Wk2], axis=1)  # [512, 128]
    wq2d = np.concatenate([Wq2, Wq2], axis=1)

    def fill(width, table, parts, dt):
        buf = np.zeros((128, width), dt)
        for name, arr in parts.items():
            off, w = table[name]
            buf[:, off:off + w] = arr.reshape(128, w).astype(dt)
        return buf

    early_base = {"wk1": pack(Wk1)}
    early2_base = {"wk2d": pack(wk2d)}
    mid_base = {"wq1": pack(Wq1), "wq2d": pack(wq2d), "wv": pack(Wv)}
    f32a_arr = fill(F32A_W, F32A, {
        "bk1": bk1.reshape(4, 128).T,
        "bq1": bq1.reshape(4, 128).T,
        "bk2d": np.concatenate([bk2, bk2]).reshape(128, 1),
        "bq2d": np.concatenate([bq2, bq2]).reshape(128, 1),
        "bvb": np.broadcast_to(bv, (128, D)),
    }, f32)
    wg_packed = pack(Wg32)

    in_maps = []
    for c in range(NCORES):
        b, i = divmod(c, 4)
        t0, t1 = i * 128, (7 - i) * 128
        xb = x[b]  # [L, D]
        xTp = pack(np.ascontiguousarray(xb.T))  # [128, 4, L]
        qx = np.concatenate(
            [xb[t0:t0 + 128].T, xb[t1:t1 + 128].T], axis=1)  # [512, 256]
        tglob = np.concatenate(
            [np.arange(t0, t0 + 128), np.arange(t1, t1 + 128)])
        mask = (np.arange(L)[:, None] <= tglob[None, :])  # [L, 256]
        xplus = np.stack([xb[t0:t0 + 128], xb[t1:t1 + 128]]) + out_bias
        epsn2 = (EPS * K * (tglob.astype(f32) + 1.0)).reshape(2, 128)

        m = {
            "early_bf": fill(EARLY_BF_W, EARLY_BF,
                             {**early_base, "xTa": xTp[:, :, 0:512]}, bf16),
            "early2_bf": fill(EARLY2_BF_W, EARLY2_BF,
                              {**early2_base,
                               "xTb": xTp[:, :, 512:1024]}, bf16),
            "mid_bf": fill(MID_BF_W, MID_BF,
                           {**mid_base, "qxT": pack(qx)}, bf16),
            "f32a": f32a_arr,
            "late_bf": fill(LATE_BF_W, LATE_BF, {
                "mask": mask.reshape(NSC, 128, 256).transpose(1, 0, 2),
                "wg": wg_packed,
            }, bf16),
            "f32b": fill(F32B_W, F32B, {
                "xplus": xplus.transpose(1, 0, 2),
                "epsn2": epsn2.T,
            }, f32),
            "cw": cw,
        }
        in_maps.append(m)
    return in_maps


def run(inputs, trace=False):
    from concourse.bass_utils import run_bass_kernel_spmd

    if "nc" not in _CACHE:
        nc = _build_program()
        nc.finalize()
        _CACHE["nc"] = nc
    nc = _CACHE["nc"]
    in_maps = _host_prepare(inputs)
    res = run_bass_kernel_spmd(nc, in_maps, list(range(NCORES)), trace=trace)
    out = np.empty((B, L, D), np.float32)
    for c in range(NCORES):
        b, i = divmod(c, 4)
        oc = np.asarray(res.results[c]["out"], np.float32)
        out[b, i * 128:(i + 1) * 128] = oc[0]
        out[b, (7 - i) * 128:(8 - i) * 128] = oc[1]
    return out, res


def kernel(**inputs):
    out, _ = run(inputs, trace=False)
    return out


# revision 36
# speedup vs baseline: 1.2119x; 1.0079x over previous
"""Trainium2 Bass kernel for nn_BaselinePhasorBlock (B=2, L=1024, D=512, K=64).

Algorithm restructure: the phasor-memory cumsum
    retrieved[t,d] = Re[ sum_k e^{-i q[t,k]} * sum_{s<=t} e^{i key[s,k]} v[s,d] ]
collapses to causal attention:
    A[t,s] = cosQ[t]·cosK[s] + sinQ[t]·sinK[s]   (dot over k)
    retrieved = tril(A) @ value
so nothing of size (L,K,D) is ever materialized.

LayerNorm folding (exact):
    LN(retrieved/norm) @ Wo + bo + x
  = scale_t * (r @ Wg - mu_t * cw) + [x + ln_b@Wo + bo]
with Wg = diag(ln_g)@Wo, cw = colsums(Wg), scale_t = 1/sqrt(var_r + eps*norm_t^2),
norm_t^2 = (t+1)*K.  (LN row stats are scale-invariant up to the eps term, which
is folded into the eps exactly.)

Sharding (8 cores, SPMD, no collectives): core c -> batch b = c//4, strip pair
i = c%4 owning t-strips [i*128, (i+1)*128) and [(7-i)*128, (8-i)*128).  The
pairing makes causal work uniform; causality is enforced by a per-core mask on
the score matrix (AT layout [s, t]).  Each core computes its batch's keys and
values over the full sequence (redundant across the 4 cores of a batch, but
avoids collectives entirely).

All matmuls run in bf16 (validated: ~1.7e-3 scale-relative error end-to-end
vs the fp32 jax reference), fp32 PSUM accumulation, fp32 residual/output path.

Inputs are host-packed into a handful of layout-exact mega-tensors so the
kernel issues only ~6 DMAs, ordered by first use (DMA-issue serialization on
the sync sequencer was the dominant stall in v1).
"""

import math
from contextlib import ExitStack

import numpy as np

B, L, D, K = 2, 1024, 512, 64
PI = math.pi
NCORES = 8
NSC = L // 128  # 8 s-chunks
NDC = D // 128  # 4 d-chunks
EPS = 1e-5

# flat offsets (in elements) inside the packed DMA groups
EARLY_BF = {"wk1": (0, 2048), "xTa": (2048, 2048)}
EARLY_BF_W = 4096
EARLY2_BF = {"xTb": (0, 2048), "wk2d": (2048, 512)}
EARLY2_BF_W = 2560
MID_BF = {"wq1": (0, 2048), "qxT": (2048, 1024),
          "wq2d": (3072, 512), "wv": (3584, 2048)}
MID_BF_W = 5632
F32A = {"bk1": (0, 4), "bq1": (4, 4), "bk2d": (8, 1), "bq2d": (9, 1),
        "bvb": (10, 512)}
F32A_W = 522
LATE_BF = {"mask": (0, 2048), "wg": (2048, 2048)}
LATE_BF_W = 4096
F32B = {"xplus": (0, 1024), "epsn2": (1024, 2)}
F32B_W = 1026

_CACHE = {}


def _build_program(gelu_override=None):
    import concourse.bacc as bacc
    import concourse.mybir as mybir
    import concourse.tile as tile

    AF = mybir.ActivationFunctionType
    ALU = mybir.AluOpType
    GELU = AF.Gelu if gelu_override is None else gelu_override
    FP32 = mybir.dt.float32
    BF16 = mybir.dt.bfloat16

    nc = bacc.Bacc()

    d_early = nc.declare_dram_parameter("early_bf", [128, EARLY_BF_W], BF16, False)
    d_early2 = nc.declare_dram_parameter("early2_bf", [128, EARLY2_BF_W], BF16, False)
    d_f32a = nc.declare_dram_parameter("f32a", [128, F32A_W], FP32, False)
    d_mid = nc.declare_dram_parameter("mid_bf", [128, MID_BF_W], BF16, False)
    d_late = nc.declare_dram_parameter("late_bf", [128, LATE_BF_W], BF16, False)
    d_f32b = nc.declare_dram_parameter("f32b", [128, F32B_W], FP32, False)
    d_cw = nc.declare_dram_parameter("cw", [1, D], BF16, False)
    d_out = nc.declare_dram_parameter("out", [2, 128, D], FP32, True)

    with tile.TileContext(nc) as tc, ExitStack() as ctx:
        consts = ctx.enter_context(tc.tile_pool(name="consts", bufs=1))
        work = ctx.enter_context(tc.tile_pool(name="work", bufs=1))
        atm_pool = ctx.enter_context(tc.tile_pool(name="atm", bufs=4))
        small = ctx.enter_context(tc.tile_pool(name="small", bufs=1))
        ps_big = ctx.enter_context(tc.tile_pool(name="ps_big", bufs=3, space="PSUM"))
        ps_at = ctx.enter_context(tc.tile_pool(name="ps_at", bufs=3, space="PSUM"))
        ps_rt = ctx.enter_context(tc.tile_pool(name="ps_rt", bufs=1, space="PSUM"))

        # ---- SBUF tiles for the packed input groups + views ----
        early = consts.tile([128, EARLY_BF_W], BF16)
        early2 = consts.tile([128, EARLY2_BF_W], BF16)
        mid = consts.tile([128, MID_BF_W], BF16)
        f32a = consts.tile([128, F32A_W], FP32)
        late = consts.tile([128, LATE_BF_W], BF16)
        f32b = consts.tile([128, F32B_W], FP32)
        cw = consts.tile([1, D], BF16)
        ones = consts.tile([128, 1], BF16)
        cosbias = consts.tile([128, 1], FP32)
        sinscale = consts.tile([128, 1], FP32)

        def view(tile_, table, name, c=None):
            off, w = table[name]
            v = tile_[:, off:off + w]
            if c is not None:
                v = v.rearrange("p (c f) -> p c f", c=c)
            return v

        wk1 = view(early, EARLY_BF, "wk1", 4)     # [128, 4, 512]
        wk2d = view(early2, EARLY2_BF, "wk2d", 4)  # [128, 4, 128]
        xTa = view(early, EARLY_BF, "xTa", 4)     # [128, 4, 512]
        xTb = view(early2, EARLY2_BF, "xTb", 4)
        wq1 = view(mid, MID_BF, "wq1", 4)
        qxT = view(mid, MID_BF, "qxT", 4)         # [128, 4, 256]
        wq2d = view(mid, MID_BF, "wq2d", 4)
        wv = view(mid, MID_BF, "wv", 4)
        bk1 = view(f32a, F32A, "bk1")             # [128, 4]
        bq1 = view(f32a, F32A, "bq1")
        bk2d = view(f32a, F32A, "bk2d")           # [128, 1]
        bq2d = view(f32a, F32A, "bq2d")
        bvb = view(f32a, F32A, "bvb")             # [128, 512]
        maskt = view(late, LATE_BF, "mask", NSC)  # [128, 8, 256]
        wg = view(late, LATE_BF, "wg", 4)
        xplus = view(f32b, F32B, "xplus", 2)      # [128, 2, 512]
        epsn2 = view(f32b, F32B, "epsn2")         # [128, 2]

        def xT(c, lo, hi):
            """x^T slice [din-chunk c, seq cols lo:hi] across the a/b halves."""
            if hi <= 512:
                return xTa[:, c, lo:hi]
            return xTb[:, c, lo - 512:hi - 512]

        # ---- DMAs: one per group, issued in need-order on sync ----
        nc.sync.dma_start(out=early, in_=d_early[:])
        nc.sync.dma_start(out=f32a, in_=d_f32a[:])
        nc.sync.dma_start(out=early2, in_=d_early2[:])
        nc.sync.dma_start(out=mid, in_=d_mid[:])
        nc.sync.dma_start(out=late, in_=d_late[:])
        nc.sync.dma_start(out=f32b, in_=d_f32b[:])
        nc.sync.dma_start(out=cw, in_=d_cw[:])
        nc.vector.memset(ones, 1.0)
        nc.vector.memset(cosbias[0:64, :], PI / 2)
        nc.vector.memset(cosbias[64:128, :], 0.0)
        nc.vector.memset(sinscale[0:64, :], -PI)
        nc.vector.memset(sinscale[64:128, :], PI)

        # ---- working SBUF tiles ----
        hkT = work.tile([128, 4, L], BF16)      # gelu(x@Wk1+b) transposed
        hqT = work.tile([128, 4, 256], BF16)
        kph2 = work.tile([128, L], BF16)        # tanh phase, duplicated halves
        qph2 = work.tile([128, 256], BF16)
        KS = work.tile([128, L], BF16)          # rows 0:64 cosK, 64:128 sinK
        QS = work.tile([128, 256], BF16)
        value = work.tile([128, NSC, D], BF16)  # value rows [s,d] per s-chunk
        rT_sb = work.tile([128, 4, 256], BF16)  # retrievedT [d, t]
        rsq = work.tile([128, 4, 256], BF16)
        out_sb = work.tile([128, 2, D], FP32)

        # ---- MLP1 (key): hkT[j, m] = gelu(Wk1^T @ xT + bk1) ----
        # per-(j,m) 1-bank groups; all m0 first (needs only the first DMA)
        for m in range(2):
            for j in range(4):
                ps = ps_big.tile([128, 512], FP32, tag="mlp")
                for c in range(4):
                    nc.tensor.matmul(
                        ps,
                        lhsT=wk1[:, c, j * 128:(j + 1) * 128],
                        rhs=xT(c, m * 512, (m + 1) * 512),
                        start=(c == 0),
                        stop=(c == 3),
                    )
                nc.scalar.activation(
                    out=hkT[:, j, m * 512:(m + 1) * 512], in_=ps,
                    func=GELU, bias=bk1[:, j:j + 1], scale=1.0,
                )

        # ---- MLP1 (query): per-j groups ----
        for j in range(4):
            ps = ps_big.tile([128, 512], FP32, tag="mlp")
            for c in range(4):
                nc.tensor.matmul(
                    ps[:, 0:256],
                    lhsT=wq1[:, c, j * 128:(j + 1) * 128],
                    rhs=qxT[:, c, :],
                    start=(c == 0),
                    stop=(c == 3),
                )
            nc.scalar.activation(
                out=hqT[:, j, :],
                in_=ps[:, 0:256],
                func=GELU, bias=bq1[:, j:j + 1], scale=1.0,
            )

        # ---- key phase matmul + tanh (duplicated halves via doubled Wk2) ----
        for m in range(2):
            ps_k = ps_big.tile([128, 512], FP32, tag="mlp")
            for j in range(4):
                nc.tensor.matmul(
                    ps_k,
                    lhsT=wk2d[:, j, :],
                    rhs=hkT[:, j, m * 512:(m + 1) * 512],
                    start=(j == 0),
                    stop=(j == 3),
                )
            nc.scalar.activation(out=kph2[:, m * 512:(m + 1) * 512],
                                 in_=ps_k, func=AF.Tanh, bias=bk2d, scale=1.0)
        # ---- query phase matmul + tanh ----
        ps_p = ps_big.tile([128, 512], FP32, tag="mlp")
        for j in range(4):
            nc.tensor.matmul(
                ps_p[:, 0:256],
                lhsT=wq2d[:, j, :],
                rhs=hqT[:, j, :],
                start=(j == 0),
                stop=(j == 3),
            )
        nc.scalar.activation(out=qph2, in_=ps_p[:, 0:256], func=AF.Tanh,
                             bias=bq2d, scale=1.0)

        # ---- |t| on the cos half (ACT Abs; in every table set) ----
        nc.scalar.activation(out=kph2[0:64, :], in_=kph2[0:64, :], func=AF.Abs)
        nc.scalar.activation(out=qph2[0:64, :], in_=qph2[0:64, :], func=AF.Abs)

        # ---- value rows: value[s, d] = x@Wv + bv ----
        # own PSUM slots (at-pool, idle this early) so value never contends
        # with the phase matmuls for mlp-pool slots (priority inversion).
        for sc in range(NSC):
            ps = ps_at.tile([128, 512], FP32, tag="at")
            for c in range(4):
                nc.tensor.matmul(
                    ps,
                    lhsT=xT(c, sc * 128, (sc + 1) * 128),
                    rhs=wv[:, c, :],
                    start=(c == 0),
                    stop=(c == 3),
                )
            nc.vector.tensor_add(out=value[:, sc, :], in0=ps, in1=bvb)

        # ---- cos/sin of phases (stacked halves: 0:64 cos, 64:128 sin) ----
        # ACT Sin domain is [-pi, pi]; with t in (-1,1):
        #   cos(pi t) = sin(pi/2 - pi |t|),  sin(pi t) = sin(pi t)
        # one Sin pass with per-partition scale (-pi / +pi) and bias (pi/2 / 0).
        nc.scalar.activation(out=KS, in_=kph2, func=AF.Sin,
                             bias=cosbias, scale=sinscale)
        nc.scalar.activation(out=QS, in_=qph2, func=AF.Sin,
                             bias=cosbias, scale=sinscale)

        # ---- scores + causal mask + retrievedT accumulation ----
        rt_ps = ps_rt.tile([128, 4, 256], FP32)
        for sc in range(NSC):
            at_ps = ps_at.tile([128, 256], FP32, tag="at")
            nc.tensor.matmul(
                at_ps,
                lhsT=KS[:, sc * 128:(sc + 1) * 128],
                rhs=QS,
                start=True,
                stop=True,
            )
            atm = atm_pool.tile([128, 256], BF16, tag="atm")
            nc.vector.tensor_mul(out=atm, in0=at_ps, in1=maskt[:, sc, :])
            # rt_ps spans 2 banks (dc 0,1 | dc 2,3): exactly one start per
            # bank (first matmul into it), one stop on each bank's last.
            for dc in range(NDC):
                nc.tensor.matmul(
                    rt_ps[:, dc, :],
                    lhsT=value[:, sc, dc * 128:(dc + 1) * 128],
                    rhs=atm,
                    start=(sc == 0 and dc in (0, 2)),
                    stop=(sc == NSC - 1 and dc in (1, 3)),
                )

        # ---- retrievedT -> SBUF (ACT, idle by now) + squares (DVE) ----
        for dc in range(NDC):
            nc.scalar.copy(out=rT_sb[:, dc, :], in_=rt_ps[:, dc, :])
        for dc in range(NDC):
            nc.vector.tensor_mul(out=rsq[:, dc, :], in0=rT_sb[:, dc, :],
                                 in1=rT_sb[:, dc, :])

        # ---- row stats: sums/sumsq share one bank (one group); row sums
        # reuse the at-pool bank (free by now) with their own group ----
        sums_ps = ps_at.tile([128, 4], FP32, tag="at")  # [sum s0, sum s1, sq s0, sq s1]
        row_ps = ps_at.tile([1, 256], FP32, tag="at")
        first = True
        n = 0
        for st in range(2):
            for src, col in ((rT_sb, st), (rsq, 2 + st)):
                for dc in range(NDC):
                    n += 1
                    nc.tensor.matmul(
                        sums_ps[:, col:col + 1],
                        lhsT=src[:, dc, st * 128:(st + 1) * 128],
                        rhs=ones,
                        start=first,
                        stop=(n == 16),
                    )
                    first = False
        for dc in range(NDC):
            nc.tensor.matmul(
                row_ps,
                lhsT=ones,
                rhs=rT_sb[:, dc, :],
                start=(dc == 0),
                stop=(dc == 3),
            )

        # negmu_row = -(row sums)/D  (bf16, feeds the rank-1 mean-fold matmul)
        negmu = small.tile([1, 256], BF16)
        nc.vector.tensor_scalar_mul(out=negmu, in0=row_ps, scalar1=-1.0 / D)

        # per-strip scale_t = 1/sqrt(var + eps*norm^2)
        mu = small.tile([128, 2], FP32)
        musq = small.tile([128, 2], FP32)
        var = small.tile([128, 2], FP32)
        scl = small.tile([128, 2], FP32)
        for st in range(2):
            nc.vector.tensor_scalar_mul(out=mu[:, st:st + 1],
                                        in0=sums_ps[:, st:st + 1],
                                        scalar1=1.0 / D)
            nc.vector.tensor_mul(out=musq[:, st:st + 1],
                                 in0=mu[:, st:st + 1], in1=mu[:, st:st + 1])
            nc.vector.scalar_tensor_tensor(
                out=var[:, st:st + 1],
                in0=sums_ps[:, 2 + st:3 + st],
                scalar=1.0 / D,
                in1=musq[:, st:st + 1],
                op0=ALU.mult,
                op1=ALU.subtract,
            )
        for st in range(2):
            nc.scalar.activation(out=scl[:, st:st + 1], in_=var[:, st:st + 1],
                                 func=AF.Sqrt, bias=epsn2[:, st:st + 1],
                                 scale=1.0)
            nc.vector.reciprocal(out=scl[:, st:st + 1], in_=scl[:, st:st + 1])

        # ---- output: out = scale * (rT^T @ Wg - mu*cw) + xplus ----
        for st in range(2):
            ps = ps_big.tile([128, 512], FP32, tag="mlp")
            reg = ps
            for dc in range(NDC):
                nc.tensor.matmul(
                    reg,
                    lhsT=rT_sb[:, dc, st * 128:(st + 1) * 128],
                    rhs=wg[:, dc, :],
                    start=(dc == 0),
                    stop=False,
                )
            nc.tensor.matmul(
                reg,
                lhsT=negmu[:, st * 128:(st + 1) * 128],
                rhs=cw,
                start=False,
                stop=True,
            )
            nc.vector.scalar_tensor_tensor(
                out=out_sb[:, st, :],
                in0=reg,
                scalar=scl[:, st:st + 1],
                in1=xplus[:, st, :],
                op0=ALU.mult,
                op1=ALU.add,
            )
            nc.sync.dma_start(out=d_out[st], in_=out_sb[:, st, :])

    return nc


def _host_prepare(inputs):
    """Build the 8 per-core input maps (host-side numpy packing)."""
    import ml_dtypes

    bf16 = ml_dtypes.bfloat16
    f32 = np.float32

    x = np.asarray(inputs["x"], f32)
    Wk1 = np.asarray(inputs["Wk1"], f32)
    bk1 = np.asarray(inputs["bk1"], f32)
    Wk2 = np.asarray(inputs["Wk2"], f32)
    bk2 = np.asarray(inputs["bk2"], f32)
    Wq1 = np.asarray(inputs["Wq1"], f32)
    bq1 = np.asarray(inputs["bq1"], f32)
    Wq2 = np.asarray(inputs["Wq2"], f32)
    bq2 = np.asarray(inputs["bq2"], f32)
    Wv = np.asarray(inputs["Wv"], f32)
    bv = np.asarray(inputs["bv"], f32)
    ln_g = np.asarray(inputs["ln_g"], f32)
    ln_b = np.asarray(inputs["ln_b"], f32)
    Wo = np.asarray(inputs["Wo"], f32)
    bo = np.asarray(inputs["bo"], f32)

    Wg32 = ln_g[:, None] * Wo
    cw = Wg32.astype(bf16).astype(f32).sum(axis=0).astype(bf16).reshape(1, D)
    out_bias = (ln_b @ Wo + bo).astype(f32)

    def pack(w):  # [D_in, F] -> [128, 4, F]
        return w.reshape(4, 128, -1).transpose(1, 0, 2)

    wk2d = np.concatenate([Wk2, Wk2], axis=1)  # [512, 128]
    wq2d = np.concatenate([Wq2, Wq2], axis=1)

    def fill(width, table, parts, dt):
        buf = np.zeros((128, width), dt)
        for name, arr in parts.items():
            off, w = table[name]
            buf[:, off:off + w] = arr.reshape(128, w).astype(dt)
        return buf

    early_base = {"wk1": pack(Wk1)}
    early2_base = {"wk2d": pack(wk2d)}
    mid_base = {"wq1": pack(Wq1), "wq2d": pack(wq2d), "wv": pack(Wv)}
    f32a_arr = fill(F32A_W, F32A, {
        "bk1": bk1.reshape(4, 128).T,
        "bq1": bq1.reshape(4, 128).T,
        "bk2d": np.concatenate([bk2, bk2]).reshape(128, 1),
        "bq2d": np.concatenate([bq2, bq2]).reshape(128, 1),
        "bvb": np.broadcast_to(bv, (128, D)),
    }, f32)
    wg_packed = pack(Wg32)

    in_maps = []
    for c in range(NCORES):
        b, i = divmod(c, 4)
        t0, t1 = i * 128, (7 - i) * 128
        xb = x[b]  # [L, D]
        xTp = pack(np.ascontiguousarray(xb.T))  # [128, 4, L]
        qx = np.concatenate(
            [xb[t0:t0 + 128].T, xb[t1:t1 + 128].T], axis=1)  # [512, 256]
        tglob = np.concatenate(
            [np.arange(t0, t0 + 128), np.arange(t1, t1 + 128)])
        mask = (np.arange(L)[:, None] <= tglob[None, :])  # [L, 256]
        xplus = np.stack([xb[t0:t0 + 128], xb[t1:t1 + 128]]) + out_bias
        epsn2 = (EPS * K * (tglob.astype(f32) + 1.0)).reshape(2, 128)

        m = {
            "early_bf": fill(EARLY_BF_W, EARLY_BF,
                             {**early_base, "xTa": xTp[:, :, 0:512]}, bf16),
            "early2_bf": fill(EARLY2_BF_W, EARLY2_BF,
                              {**early2_base,
                               "xTb": xTp[:, :, 512:1024]}, bf16),
            "mid_bf": fill(MID_BF_W, MID_BF,
                           {**mid_base, "qxT": pack(qx)}, bf16),
            "f32a": f32a_arr,
            "late_bf": fill(LATE_BF_W, LATE_BF, {
                "mask": mask.reshape(NSC, 128, 256).transpose(1, 0, 2),
                "wg": wg_packed,
            }, bf16),
            "f32b": fill(F32B_W, F32B, {
                "xplus": xplus.transpose(1, 0, 2),
                "epsn2": epsn2.T,
            }, f32),
            "cw": cw,
        }
        in_maps.append(m)
    return in_maps


def run(inputs, trace=False):
    from concourse.bass_utils import run_bass_kernel_spmd

    if "nc" not in _CACHE:
        nc = _build_program()
        nc.finalize()
        _CACHE["nc"] = nc
    nc = _CACHE["nc"]
    in_maps = _host_prepare(inputs)
    res = run_bass_kernel_spmd(nc, in_maps, list(range(NCORES)), trace=trace)
    out = np.empty((B, L, D), np.float32)
    for c in range(NCORES):
        b, i = divmod(c, 4)
        oc = np.asarray(res.results[c]["out"], np.float32)
        out[b, i * 128:(i + 1) * 128] = oc[0]
        out[b, (7 - i) * 128:(8 - i) * 128] = oc[1]
    return out, res


def kernel(**inputs):
    out, _ = run(inputs, trace=False)
    return out
